# revision 1
# baseline (speedup 1.0000x reference)
"""Multi-head attention forward on 8 TRN2 NeuronCores (data-parallel over batch).

Reference computation (B=64, T=197, D=768, H=12, DK=64, fp32):
    q = split_heads(x @ Wq + bq); k = ...; v = ...
    scores = floor((q @ k^T) / 8); attn = softmax(scores); out = attn @ v
    return merge_heads(out) @ Wo + bo

Numerics: floor() before softmax makes the Q/K path extremely sensitive.
The q/k projections run as error-corrected fp16 matmuls: each fp32
operand a is split as a = a_hi + a_lo (fp16 halves) and
a@b = a_hi@b_hi + a_hi@b_lo + a_lo@b_hi (the lo@lo term is negligible).
Products are exact in fp32 PSUM, so this is slightly MORE accurate than
native fp32 matmul (validated on HW: 2.6e-5 vs 6e-5 abs err vs fp64) at
3 cycles/row instead of 4.  The scores matmul stays native fp32 (2-pass,
4 cyc/row): multi-matmul accumulation groups interleaved across PE row
groups fault the HW, so the 3-term scheme cannot be row-packed there.
The V path (v proj, attn@v, out proj) runs in plain fp16 (1 cyc/row).

Per-core dataflow (8 batch elements each, all-transposed activations):
  P0:  x row-chunks DMA'd, PE-transposed; DVE splits them into
       x_hi/x_lo fp16 tiles [128,1576].
  P1a: q = (Wq/8)^T @ x, k = Wk^T @ x via 3-term fp16 (weights pre-split
       on host); PSUM fp32 results copied to fp32 qT/kT (ScalarE).
  P1b: v16e[b,kc][keys,12*65] = x_hi^T @ Wv16 (fp16), heads strided by
       65 with a ones column per head (memset first) so attn@v also
       produces the softmax denominator.
  P2 (per b, head-pair): scoresT[keys,197] = kT.T @ qT (fp32, 2 heads
      row-packed via tile_position, separate PSUM tiles; both key chunks
      share one tile column-wise); floor via round_half_even(x-0.5)
      (DVE magic-number add, magic 1.5*2^23) with the -magic correction
      folded into the ScalarE Exp bias; attn@v transposed ->
      outT[0:64]=out, outT[64]=denominator. Denominators for all 12
      heads are gathered into one tile (partition base 32*(h%4), col
      block h//4 -- DVE writes must start at partition 0/32/64/96),
      one batched reciprocal, then per-pair PE-broadcast + DVE multiply.
  P3 (per b): final = outT16^T @ Wo16, DVE copy to SBUF, DMA out.

Bias matmuls (K=1 ones-row) are only emitted when any bias is nonzero;
the build is specialized on that flag.  All PSUM tiles come from one
shared-tag pool (8 banks round-robin) so phases overlap freely.
"""

import numpy as np
import ml_dtypes

B, T, D, H, DK = 64, 197, 768, 12, 64
NCORES = 8
BL = B // NCORES          # 8 batch elements per core
R = BL * T                # 1576 rows per core
ND = D // 128             # 6 chunks of 128 along D
NC4 = 4                   # proj col chunks
CW = R // NC4             # 394
HV = DK + 1               # 65: per-head v stride (ones column at 64)
ROWCHUNKS = [(i * 128, min(128, R - i * 128)) for i in range((R + 127) // 128)]
KEYCHUNKS = [(0, 128), (128, 69)]
MAGIC = float(3 * 2 ** 22)  # 1.5*2^23: x-0.5+MAGIC stays in [2^23,2^24), ulp=1

_CACHE = {}


def _build(has_bias):
    import concourse.bacc as bacc
    import concourse.mybir as mybir
    import concourse.tile as tile
    from concourse.masks import make_identity

    f32 = mybir.dt.float32
    f16 = mybir.dt.float16
    AF = mybir.ActivationFunctionType
    OP = mybir.AluOpType

    nc = bacc.Bacc("TRN2", target_bir_lowering=False, debug=False,
                   num_devices=NCORES)

    x_d = nc.dram_tensor("x", [R, D], f32, kind="ExternalInput").ap()
    w_d = {}
    for nm in ("wq_hi", "wq_lo", "wk_hi", "wk_lo", "wv", "wo"):
        w_d[nm] = nc.dram_tensor(nm, [D, D], f16, kind="ExternalInput").ap()
    if has_bias:
        bq_d = nc.dram_tensor("bq", [1, D], f16, kind="ExternalInput").ap()
        bk_d = nc.dram_tensor("bk", [1, D], f16, kind="ExternalInput").ap()
        bv_d = nc.dram_tensor("bv", [1, D], f16, kind="ExternalInput").ap()
        bo_d = nc.dram_tensor("bo", [1, D], f16, kind="ExternalInput").ap()
    out_d = nc.dram_tensor("out", [R, D], f32, kind="ExternalOutput").ap()

    with tile.TileContext(nc) as tc:
        with tc.tile_pool(name="static", bufs=1) as Ps, \
             tc.tile_pool(name="psum", bufs=8, space="PSUM") as Pp:

            def ptile(nm):
                return Pp.tile([128, CW], f32, name=nm, tag="ps", bufs=8,
                               uniquify=True)

            qT = [Ps.tile([128, R], f32, name=f"qT{i}") for i in range(ND)]
            kT = [Ps.tile([128, R], f32, name=f"kT{i}") for i in range(ND)]
            # v16e[2b+kc][keys<=128, 12*65]; col h*65+64 holds ones
            v16e = [Ps.tile([128, H * HV], f16, name=f"v16e_{i}")
                    for i in range(2 * BL)]
            ones_row = Ps.tile([128, CW], f16, name="ones_row")
            id32 = Ps.tile([128, 128], f32, name="id32")
            negmagic = Ps.tile([128, 1], f32, name="negmagic")

            nc.vector.memset(ones_row, 1.0)
            nc.vector.memset(negmagic, -MAGIC)
            make_identity(nc, id32)
            if has_bias:
                bq_sb = Ps.tile([1, D], f16, name="bq_sb")
                bk_sb = Ps.tile([1, D], f16, name="bk_sb")
                bv_sb = Ps.tile([1, D], f16, name="bv_sb")
                bo_sb = Ps.tile([1, D], f16, name="bo_sb")
                nc.sync.dma_start(bq_sb, bq_d)
                nc.sync.dma_start(bk_sb, bk_d)
                nc.sync.dma_start(bv_sb, bv_d)
                nc.sync.dma_start(bo_sb, bo_d)

            # ---------------- P0 + P1a: x split, q/k projections -----------
            with tc.tile_pool(name="ph1", bufs=1) as P1:
                xhi = [P1.tile([128, R], f16, name=f"xhi{i}") for i in range(ND)]
                xlo = [P1.tile([128, R], f16, name=f"xlo{i}") for i in range(ND)]
                with tc.tile_pool(name="wqk", bufs=1) as Pw:
                    wsb = {nm: [Pw.tile([128, D], f16, name=f"{nm}{k}")
                                for k in range(ND)]
                           for nm in ("wq_hi", "wq_lo", "wk_hi", "wk_lo")}
                    for nm, tiles in wsb.items():
                        for k in range(ND):
                            nc.sync.dma_start(
                                tiles[k], w_d[nm][k * 128:(k + 1) * 128, :])

                    # P0: transpose x chunks, split into hi/lo fp16
                    with tc.tile_pool(name="xst", bufs=5) as Pst:
                        for (roff, rn) in ROWCHUNKS:
                            xs = Pst.tile([128, D], f32, name="xs", tag="xs")
                            nc.sync.dma_start(xs[:rn, :384],
                                              x_d[roff:roff + rn, :384])
                            nc.sync.dma_start(xs[:rn, 384:],
                                              x_d[roff:roff + rn, 384:])
                            for d in range(ND):
                                tp = ptile("tp")
                                nc.tensor.transpose(
                                    tp[:128, :rn],
                                    xs[:rn, d * 128:(d + 1) * 128],
                                    id32[:rn, :rn])
                                hi = xhi[d][:, roff:roff + rn]
                                nc.vector.tensor_copy(hi, tp[:128, :rn])
                                nc.vector.tensor_tensor(
                                    xlo[d][:, roff:roff + rn],
                                    tp[:128, :rn], hi, OP.subtract)

                    # P1a: q/k projections, 3-term fp16; split outputs
                    for (whi, wlo, b_nm, dst) in (
                            ("wq_hi", "wq_lo", "bq", qT),
                            ("wk_hi", "wk_lo", "bk", kT)):
                        for n in range(ND):
                            ns = slice(n * 128, (n + 1) * 128)
                            for c in range(NC4):
                                cs = slice(c * CW, (c + 1) * CW)
                                pp = ptile("pp")
                                for k in range(ND):
                                    nc.tensor.matmul(
                                        pp, wsb[whi][k][:, ns], xhi[k][:, cs],
                                        start=(k == 0), stop=False)
                                    nc.tensor.matmul(
                                        pp, wsb[whi][k][:, ns], xlo[k][:, cs],
                                        start=False, stop=False)
                                    nc.tensor.matmul(
                                        pp, wsb[wlo][k][:, ns], xhi[k][:, cs],
                                        start=False,
                                        stop=(k == ND - 1 and not has_bias))
                                if has_bias:
                                    bsb = {"bq": bq_sb, "bk": bk_sb}[b_nm]
                                    nc.tensor.matmul(
                                        pp, bsb[:1, ns], ones_row[:1, :CW],
                                        start=False, stop=True)
                                nc.scalar.activation(dst[n][:, cs], pp,
                                                     AF.Copy)

                # P1b: v projection (fp16, from xhi)
                with tc.tile_pool(name="ph1b", bufs=1) as P1b:
                    wv_sb = [P1b.tile([128, D], f16, name=f"wv_sb{k}")
                             for k in range(ND)]
                    for k in range(ND):
                        nc.sync.dma_start(wv_sb[k],
                                          w_d["wv"][k * 128:(k + 1) * 128, :])
                    for i in range(2 * BL):
                        nc.vector.memset(v16e[i], 1.0)

                    for b in range(BL):
                        base = b * T
                        for kc, (koff, klen) in enumerate(KEYCHUNKS):
                            dst = v16e[2 * b + kc]
                            dst3 = dst[:klen, :].rearrange(
                                "p (h c) -> p h c", c=HV)[:, :, 0:DK]
                            for half in range(2):
                                c0 = half * 384
                                vp = ptile("vp")
                                vps = vp[:klen, :384]
                                for d in range(ND):
                                    nc.tensor.matmul(
                                        vps,
                                        xhi[d][:, base + koff:
                                               base + koff + klen],
                                        wv_sb[d][:, c0:c0 + 384],
                                        start=(d == 0),
                                        stop=(d == ND - 1 and not has_bias))
                                if has_bias:
                                    nc.tensor.matmul(
                                        vps, ones_row[:1, :klen],
                                        bv_sb[:1, c0:c0 + 384],
                                        start=False, stop=True)
                                nc.scalar.activation(
                                    dst3[:, half * 6:(half + 1) * 6, :],
                                    vps.rearrange("p (h c) -> p h c", c=DK),
                                    AF.Copy)

            # ---------------- P2 + P3 fused per batch element --------------
            with tc.tile_pool(name="ph23", bufs=1) as P23, \
                 tc.tile_pool(name="att_sb", bufs=1) as Pat:
                wo_sb = [P23.tile([128, D], f16, name=f"wo_sb{k}")
                         for k in range(ND)]
                for k in range(ND):
                    nc.sync.dma_start(wo_sb[k],
                                      w_d["wo"][k * 128:(k + 1) * 128, :])

                def attn_stage(b):
                    base = b * T
                    oT16 = [Pat.tile([128, T], f16, name=f"oT16_{b}_{hp}",
                                     tag="oT16", bufs=3 * ND + 4)
                            for hp in range(ND)]
                    # denominators: head h at partition 32*(h%4), col h//4
                    dn = Pat.tile([128, 3 * T], f32, name="dn", tag="dn",
                                  bufs=3)
                    otfs = []
                    for hp in range(ND):
                        # --- scoresT (fp32, 2 heads row-packed); both key
                        # chunks share one PSUM tile (same row group ->
                        # sequential MMs, disjoint column ranges) ---
                        eT = []
                        for hl in range(2):
                            pb = 64 * hl
                            qs = slice(base, base + T)
                            sc = ptile("sc")
                            for kc, (koff, klen) in enumerate(KEYCHUNKS):
                                ks = slice(base + koff, base + koff + klen)
                                nc.tensor.matmul(
                                    sc[:klen, kc * T:(kc + 1) * T],
                                    kT[hp][pb:pb + 64, ks],
                                    qT[hp][pb:pb + 64, qs],
                                    start=True, stop=True,
                                    tile_position=(pb, 0))
                            fl = Pat.tile([128, 2 * T], f32, name="fl",
                                          tag="fl", bufs=6)
                            nc.vector.tensor_scalar(
                                fl, sc, -0.5, MAGIC, OP.add, OP.add)
                            e_t = Pat.tile([128, 2 * T], f16, name="e_t",
                                           tag="eT", bufs=8)
                            nc.scalar.activation(
                                e_t, fl, AF.Exp, bias=negmagic[:, :1])
                            eT.append(e_t)

                        # --- attn @ v (fp16); col 64 = ones -> denominator.
                        # Both heads share one PSUM tile (same row groups ->
                        # sequential MMs, disjoint column ranges). ---
                        otf = Pat.tile([128, T], f32, name="otf", tag="otf",
                                       bufs=3 * ND + 2)
                        op_ = ptile("oT")
                        for hl in range(2):
                            h = 2 * hp + hl
                            for kc, (koff, klen) in enumerate(KEYCHUNKS):
                                nc.tensor.matmul(
                                    op_[0:HV, hl * T:(hl + 1) * T],
                                    v16e[2 * b + kc][:klen,
                                                     h * HV:(h + 1) * HV],
                                    eT[hl][:klen, kc * T:(kc + 1) * T],
                                    start=(kc == 0),
                                    stop=(kc == len(KEYCHUNKS) - 1))
                            pbase = 32 * (h % 4)
                            cb = (h // 4) * T
                            nc.vector.tensor_copy(
                                dn[pbase:pbase + 1, cb:cb + T],
                                op_[64:65, hl * T:(hl + 1) * T])
                            if hl == 0:
                                nc.scalar.activation(otf[0:64, :],
                                                     op_[0:64, :T], AF.Copy)
                            else:
                                nc.vector.tensor_copy(otf[64:128, :],
                                                      op_[0:64, T:2 * T])
                        otfs.append(otf)
                    return oT16, dn, otfs

                def norm_final_stage(b, oT16, dn, otfs):
                    base = b * T
                    # --- normalize: batched recip -> PE broadcast -> mul ---
                    rdf = Pat.tile([128, 3 * T], f32, name="rdf", tag="rdf",
                                   bufs=3)
                    rd16 = Pat.tile([128, 3 * T], f16, name="rd16",
                                    tag="rd16", bufs=3)
                    nc.vector.reciprocal_approx_fast(rdf, dn)
                    nc.vector.tensor_copy(rd16, rdf)
                    for hp in range(ND):
                        bc = ptile("bc")
                        for hl in range(2):
                            h = 2 * hp + hl
                            pbase = 32 * (h % 4)
                            cb = (h // 4) * T
                            nc.tensor.matmul(
                                bc[64 * hl:64 * hl + 64, :T],
                                ones_row[pbase:pbase + 1, :64],
                                rd16[pbase:pbase + 1, cb:cb + T],
                                start=True, stop=True,
                                tile_position=(pbase, 64 * hl))
                        nc.vector.tensor_tensor(oT16[hp], otfs[hp],
                                                bc[:, :T], OP.mult)

                    # --- P3: final projection (+bias) + store ---
                    for (roff, rn) in ((0, 128), (128, T - 128)):
                        fs = Pat.tile([128, D], f32, name="fs", tag="fs",
                                      bufs=4)
                        for half in range(2):
                            c0 = half * 384
                            fp_ = ptile("fp")
                            for d in range(ND):
                                nc.tensor.matmul(
                                    fp_[:rn, :384],
                                    oT16[d][:, roff:roff + rn],
                                    wo_sb[d][:, c0:c0 + 384],
                                    start=(d == 0),
                                    stop=(d == ND - 1 and not has_bias))
                            if has_bias:
                                nc.tensor.matmul(
                                    fp_[:rn, :384], ones_row[:1, :rn],
                                    bo_sb[:1, c0:c0 + 384],
                                    start=False, stop=True)
                            nc.scalar.activation(fs[:rn, c0:c0 + 384],
                                                 fp_[:rn, :384], AF.Copy)
                        nc.sync.dma_start(
                            out_d[base + roff:base + roff + rn, :], fs[:rn, :])

                pending = []
                for b in range(BL):
                    st = attn_stage(b)
                    pending.append((b, st))
                    if len(pending) > 2:
                        pb_, ps_ = pending.pop(0)
                        norm_final_stage(pb_, *ps_)
                for pb_, ps_ in pending:
                    norm_final_stage(pb_, *ps_)

    nc.compile()
    return nc


def _split16(a):
    hi = a.astype(np.float16)
    lo = (a - hi.astype(np.float32)).astype(np.float16)
    return hi, lo


def _prep_weights(Wq, bq, Wk, bk, Wv, bv, Wo, bo, has_bias):
    f32 = np.float32
    wq = np.asarray(Wq, f32) * f32(0.125)
    wk = np.asarray(Wk, f32)
    wq_hi, wq_lo = _split16(wq)
    wk_hi, wk_lo = _split16(wk)
    w = {
        "wq_hi": wq_hi, "wq_lo": wq_lo,
        "wk_hi": wk_hi, "wk_lo": wk_lo,
        "wv": np.asarray(Wv, f32).astype(np.float16),
        "wo": np.asarray(Wo, f32).astype(np.float16),
    }
    if has_bias:
        w["bq"] = (np.asarray(bq, f32) * f32(0.125)).astype(
            np.float16).reshape(1, D)
        w["bk"] = np.asarray(bk, f32).astype(np.float16).reshape(1, D)
        w["bv"] = np.asarray(bv, f32).astype(np.float16).reshape(1, D)
        w["bo"] = np.asarray(bo, f32).astype(np.float16).reshape(1, D)
    return w


def kernel(x, Wq, bq, Wk, bk, Wv, bv, Wo, bo):
    from concourse import bass_utils

    has_bias = any(float(np.abs(np.asarray(v)).max()) != 0.0
                   for v in (bq, bk, bv, bo))
    key = ("nc", has_bias)
    if key not in _CACHE:
        _CACHE[key] = _build(has_bias)
    nc = _CACHE[key]

    x = np.asarray(x, np.float32)
    w = _prep_weights(Wq, bq, Wk, bk, Wv, bv, Wo, bo, has_bias)
    in_maps = []
    for c in range(NCORES):
        m = dict(w)
        m["x"] = np.ascontiguousarray(
            x[c * BL:(c + 1) * BL].reshape(R, D))
        in_maps.append(m)

    res = bass_utils.run_bass_kernel_spmd(nc, in_maps, list(range(NCORES)))
    out = np.concatenate(
        [res.results[c]["out"].reshape(BL, T, D) for c in range(NCORES)],
        axis=0)
    return out.astype(np.float32)



# revision 2
# speedup vs baseline: 1.4594x; 1.4594x over previous
"""Multi-head attention forward on 8 TRN2 NeuronCores (data-parallel over batch).

Reference computation (B=64, T=197, D=768, H=12, DK=64, fp32):
    q = split_heads(x @ Wq + bq); k = ...; v = ...
    scores = floor((q @ k^T) / 8); attn = softmax(scores); out = attn @ v
    return merge_heads(out) @ Wo + bo

Numerics: floor() before softmax makes the Q/K path sensitive.  q/k
projections run as 2-term fp16 matmuls: W is split hi+lo (22-bit
mantissa), x is truncated to fp16 (x = fp16(x), the x_lo term is
dropped): q = x16 @ W_hi + x16 @ W_lo, exact fp32 PSUM accumulation.
Measured rel err vs the fp32 reference: 1.5e-2 (budget 2e-2); the
3-term variant (adds x_lo @ W_hi, rel err 1.2e-3) is kept behind
N_TERMS=3.  The scores matmul is native fp32, two heads row-packed via
tile_position (packed pairs execute concurrently on the PE).  The V
path (v proj, attn@v, out proj) runs in plain fp16.

Schedule: the whole kernel is one software-pipelined instruction
stream so the PE never idles (idle >3.4us re-throttles the PE clock to
1.2GHz via HAM):
  - x is cast to fp16 on host and DMA'd with transpose=True straight
    into xT tiles (no on-chip transpose phase).
  - Projections are processed in 4 column chunks of 394 rows = one
    batch pair each.  Attention for batch pair p (scores -> floor
    (DVE magic-number round) -> Exp (ScalarE, -MAGIC folded into the
    activation bias) -> attn@v -> normalize -> out proj) is issued
    interleaved into projection chunk p+1's matmul stream, so DVE /
    ScalarE work overlaps dense PE work.
  - attn@v right-appends a ones column per head (v stride 65) so the
    softmax denominator falls out of the same matmul; denominators are
    gathered, one batched reciprocal, PE-broadcast, DVE multiply.
  - PE warmup matmuls + Exp-table preload run during the initial DMA
    wait; DMAs are issued in consumption order (wq, x-first-half, ...).

All PSUM tiles come from one shared-tag pool (8 banks round-robin).
"""

import numpy as np

B, T, D, H, DK = 64, 197, 768, 12, 64
NCORES = 8
BL = B // NCORES          # 8 batch elements per core
R = BL * T                # 1576 rows per core
RPAD = 1584               # padded to a multiple of 16 for DMA transpose
ND = D // 128             # 6 chunks of 128 along D
NC4 = 4                   # proj col chunks (one batch pair each)
CW = R // NC4             # 394 = 2*T
HV = DK + 1               # 65: per-head v stride (ones column at 64)
KEYCHUNKS = [(0, 128), (128, 69)]
MAGIC = float(3 * 2 ** 22)  # 1.5*2^23: x-0.5+MAGIC stays in [2^23,2^24), ulp=1
N_TERMS = 2               # 2: q/k = x16@W_hi + x16@W_lo; 3: + xlo@W_hi

_CACHE = {}


def _build(has_bias, n_terms):
    import concourse.bacc as bacc
    import concourse.mybir as mybir
    import concourse.tile as tile

    f32 = mybir.dt.float32
    f16 = mybir.dt.float16
    AF = mybir.ActivationFunctionType
    OP = mybir.AluOpType

    nc = bacc.Bacc("TRN2", target_bir_lowering=False, debug=False,
                   num_devices=NCORES)

    x16_d = nc.dram_tensor("x16", [RPAD, D], f16, kind="ExternalInput").ap()
    if n_terms == 3:
        xlo_d = nc.dram_tensor("xlo", [RPAD, D], f16,
                               kind="ExternalInput").ap()
    w_d = {}
    for nm in ("wq_hi", "wq_lo", "wk_hi", "wk_lo", "wv", "wo"):
        w_d[nm] = nc.dram_tensor(nm, [D, D], f16, kind="ExternalInput").ap()
    if has_bias:
        b_d = {nm: nc.dram_tensor(nm, [1, D], f16, kind="ExternalInput").ap()
               for nm in ("bq", "bk", "bv", "bo")}
    out_d = nc.dram_tensor("out", [R, D], f32, kind="ExternalOutput").ap()

    with tile.TileContext(nc) as tc:
        with tc.tile_pool(name="static", bufs=1) as Ps, \
             tc.tile_pool(name="work", bufs=1) as Pw, \
             tc.tile_pool(name="psum", bufs=8, space="PSUM") as Pp:

            def ptile(nm):
                return Pp.tile([128, CW], f32, name=nm, tag="ps", bufs=8,
                               uniquify=True)

            xhi = [Ps.tile([128, RPAD], f16, name=f"xhi{d}")
                   for d in range(ND)]
            if n_terms == 3:
                xlo = [Ps.tile([128, RPAD], f16, name=f"xlo{d}")
                       for d in range(ND)]
            wsb = {nm: [Ps.tile([128, D], f16, name=f"{nm}{k}")
                        for k in range(ND)]
                   for nm in ("wq_hi", "wq_lo", "wk_hi", "wk_lo", "wv", "wo")}
            # v16e[2b+kc][keys<=128, 12*65]; col h*65+64 holds ones
            v16e = [Ps.tile([128, H * HV], f16, name=f"v16e_{i}")
                    for i in range(2 * BL)]
            ones_row = Ps.tile([128, CW], f16, name="ones_row")
            negmagic = Ps.tile([128, 1], f32, name="negmagic")
            prime = Ps.tile([1, 1], f16, name="prime")
            if has_bias:
                bsb = {nm: Ps.tile([1, D], f16, name=f"{nm}_sb")
                       for nm in ("bq", "bk", "bv", "bo")}

            # ---- no-DMA-dependency setup: memsets, engine warmups ----
            nc.vector.memset(ones_row, 1.0)
            nc.vector.memset(negmagic, -MAGIC)
            for i in range(2 * BL):
                onescol = v16e[i].rearrange("p (h c) -> p h c",
                                            c=HV)[:, :, DK:DK + 1]
                nc.gpsimd.memset(onescol, 1.0)
            # Exp table preload on ScalarE (one-time 1.3us table load)
            nc.scalar.activation(prime, ones_row[:1, :1], AF.Exp,
                                 bias=negmagic[:1, :1])
            # PE warmup: keep HAM at full clock until real work arrives
            for i in range(18):
                wu = ptile("wu")
                nc.tensor.matmul(wu, ones_row[:, :128], ones_row,
                                 start=True, stop=True)

            # ---- DMAs in consumption order ----
            for k in range(ND):
                nc.sync.dma_start(wsb["wq_hi"][k],
                                  w_d["wq_hi"][k * 128:(k + 1) * 128, :])
            # x transposed straight into xhi; split rows so the first half
            # (covers proj chunks 0-1 + v proj pair 0) lands early
            XSPLIT = 800  # multiple of 16
            for d in range(ND):
                nc.sync.dma_start(xhi[d][:, :XSPLIT],
                                  x16_d[:XSPLIT, d * 128:(d + 1) * 128],
                                  transpose=True)
            for k in range(ND):
                nc.sync.dma_start(wsb["wq_lo"][k],
                                  w_d["wq_lo"][k * 128:(k + 1) * 128, :])
            for d in range(ND):
                nc.sync.dma_start(xhi[d][:, XSPLIT:],
                                  x16_d[XSPLIT:, d * 128:(d + 1) * 128],
                                  transpose=True)
            if n_terms == 3:
                for d in range(ND):
                    nc.sync.dma_start(xlo[d][:, :],
                                      xlo_d[:, d * 128:(d + 1) * 128],
                                      transpose=True)
            for nm in ("wk_hi", "wk_lo", "wv", "wo"):
                for k in range(ND):
                    nc.sync.dma_start(wsb[nm][k],
                                      w_d[nm][k * 128:(k + 1) * 128, :])
            if has_bias:
                for nm in ("bq", "bk", "bv", "bo"):
                    nc.sync.dma_start(bsb[nm], b_d[nm])

            # ---- stage helpers (each call ISSUES instructions) ----
            qT = {}   # (proj, c, n) -> sbuf tile [128, CW] f32
            eTs = {}  # (b, hp) -> [e_t hl0, e_t hl1]
            otfs = {}  # (b, hp) -> otf tile
            oT16s = {}  # (b, hp) -> oT16 tile
            dns = {}
            rd16s = {}

            def qk_tile(c, proj, n):
                whi, wlo, b_nm = (("wq_hi", "wq_lo", "bq") if proj == "q"
                                  else ("wk_hi", "wk_lo", "bk"))
                cs = slice(c * CW, (c + 1) * CW)
                ns = slice(n * 128, (n + 1) * 128)
                pp = ptile("pp")
                for k in range(ND):
                    nc.tensor.matmul(pp, wsb[whi][k][:, ns], xhi[k][:, cs],
                                     start=(k == 0), stop=False)
                for k in range(ND):
                    last = (k == ND - 1 and n_terms == 2 and not has_bias)
                    nc.tensor.matmul(pp, wsb[wlo][k][:, ns], xhi[k][:, cs],
                                     start=False, stop=last)
                if n_terms == 3:
                    for k in range(ND):
                        last = (k == ND - 1 and not has_bias)
                        nc.tensor.matmul(pp, wsb[whi][k][:, ns],
                                         xlo[k][:, cs],
                                         start=False, stop=last)
                if has_bias:
                    nc.tensor.matmul(pp, bsb[b_nm][:1, ns],
                                     ones_row[:1, :CW],
                                     start=False, stop=True)
                dst = Pw.tile([128, CW], f32, name=f"{proj}T", tag=f"{proj}T",
                              bufs=12, uniquify=True)
                nc.scalar.activation(dst, pp, AF.Copy)
                qT[(proj, c, n)] = dst

            def vp_unit(b, j):
                kc, half = j // 2, j % 2
                koff, klen = KEYCHUNKS[kc]
                base = b * T
                c0 = half * 384
                vp = ptile("vp")
                vps = vp[:klen, :384]
                for d in range(ND):
                    nc.tensor.matmul(
                        vps, xhi[d][:, base + koff:base + koff + klen],
                        wsb["wv"][d][:, c0:c0 + 384],
                        start=(d == 0),
                        stop=(d == ND - 1 and not has_bias))
                if has_bias:
                    nc.tensor.matmul(vps, ones_row[:1, :klen],
                                     bsb["bv"][:1, c0:c0 + 384],
                                     start=False, stop=True)
                dst = v16e[2 * b + kc]
                dst3 = dst[:klen, :].rearrange("p (h c) -> p h c",
                                               c=HV)[:, :, 0:DK]
                nc.scalar.activation(
                    dst3[:, half * 6:(half + 1) * 6, :],
                    vps.rearrange("p (h c) -> p h c", c=DK), AF.Copy)

            def sc_unit(b, hp):
                c = b // 2
                qoff = (b % 2) * T
                eT = []
                for hl in range(2):
                    pb = 64 * hl
                    sc = ptile("sc")
                    for kc, (koff, klen) in enumerate(KEYCHUNKS):
                        nc.tensor.matmul(
                            sc[:klen, kc * T:(kc + 1) * T],
                            qT[("k", c, hp)][pb:pb + 64,
                                             qoff + koff:qoff + koff + klen],
                            qT[("q", c, hp)][pb:pb + 64, qoff:qoff + T],
                            start=True, stop=True, tile_position=(pb, 0))
                    fl = Pw.tile([128, 2 * T], f32, name="fl", tag="fl",
                                 bufs=5, uniquify=True)
                    nc.vector.tensor_scalar(fl, sc, -0.5, MAGIC,
                                            OP.add, OP.add)
                    e_t = Pw.tile([128, 2 * T], f16, name="e_t", tag="eT",
                                  bufs=10, uniquify=True)
                    nc.scalar.activation(e_t, fl, AF.Exp,
                                         bias=negmagic[:, :1])
                    eT.append(e_t)
                eTs[(b, hp)] = eT

            def av_unit(b, hp):
                eT = eTs.pop((b, hp))
                if hp == 0:
                    dns[b] = Pw.tile([128, 3 * T], f32, name="dn", tag="dn",
                                     bufs=3, uniquify=True)
                dn = dns[b]
                otf = Pw.tile([128, T], f32, name="otf", tag="otf",
                              bufs=14, uniquify=True)
                op_ = ptile("oT")
                for hl in range(2):
                    h = 2 * hp + hl
                    for kc, (koff, klen) in enumerate(KEYCHUNKS):
                        nc.tensor.matmul(
                            op_[0:HV, hl * T:(hl + 1) * T],
                            v16e[2 * b + kc][:klen, h * HV:(h + 1) * HV],
                            eT[hl][:klen, kc * T:(kc + 1) * T],
                            start=(kc == 0), stop=(kc == len(KEYCHUNKS) - 1))
                    pbase = 32 * (h % 4)
                    cb = (h // 4) * T
                    nc.vector.tensor_copy(dn[pbase:pbase + 1, cb:cb + T],
                                          op_[64:65, hl * T:(hl + 1) * T])
                    if hl == 0:
                        nc.scalar.activation(otf[0:64, :], op_[0:64, :T],
                                             AF.Copy)
                    else:
                        nc.vector.tensor_copy(otf[64:128, :],
                                              op_[0:64, T:2 * T])
                otfs[(b, hp)] = otf

            def recip_unit(b):
                dn = dns.pop(b)
                rdf = Pw.tile([128, 3 * T], f32, name="rdf", tag="rdf",
                              bufs=2, uniquify=True)
                rd16 = Pw.tile([128, 3 * T], f16, name="rd16", tag="rd16",
                               bufs=3, uniquify=True)
                nc.vector.reciprocal_approx_fast(rdf, dn)
                nc.vector.tensor_copy(rd16, rdf)
                rd16s[b] = rd16

            def bc_unit(b, hp):
                rd16 = rd16s[b]
                bc = ptile("bc")
                for hl in range(2):
                    h = 2 * hp + hl
                    pbase = 32 * (h % 4)
                    cb = (h // 4) * T
                    nc.tensor.matmul(
                        bc[64 * hl:64 * hl + 64, :T],
                        ones_row[pbase:pbase + 1, :64],
                        rd16[pbase:pbase + 1, cb:cb + T],
                        start=True, stop=True,
                        tile_position=(pbase, 64 * hl))
                oT16 = Pw.tile([128, T], f16, name="oT16", tag="oT16",
                               bufs=14, uniquify=True)
                nc.vector.tensor_tensor(oT16, otfs.pop((b, hp)), bc[:, :T],
                                        OP.mult)
                oT16s[(b, hp)] = oT16

            def op_unit(b, rc):
                base = b * T
                roff, rn = (0, 128) if rc == 0 else (128, T - 128)
                for half in range(2):
                    c0 = half * 384
                    fp_ = ptile("fp")
                    for d in range(ND):
                        nc.tensor.matmul(
                            fp_[:rn, :384],
                            oT16s[(b, d)][:, roff:roff + rn],
                            wsb["wo"][d][:, c0:c0 + 384],
                            start=(d == 0),
                            stop=(d == ND - 1 and not has_bias))
                    if has_bias:
                        nc.tensor.matmul(fp_[:rn, :384], ones_row[:1, :rn],
                                         bsb["bo"][:1, c0:c0 + 384],
                                         start=False, stop=True)
                    fs = Pw.tile([128, 384], f32, name="fs", tag="fs",
                                 bufs=4, uniquify=True)
                    nc.scalar.activation(fs[:rn, :], fp_[:rn, :384], AF.Copy)
                    nc.sync.dma_start(
                        out_d[base + roff:base + roff + rn, c0:c0 + 384],
                        fs[:rn, :])
                if rc == 1:
                    for d in range(ND):
                        del oT16s[(b, d)]

            # ---- pipelined program ----
            # c0 prologue: projections chunk 0 + v for pair 0
            for n in range(ND):
                qk_tile(0, "q", n)
            for n in range(ND):
                qk_tile(0, "k", n)
            for j in range(4):
                vp_unit(0, j)
            for j in range(4):
                vp_unit(1, j)

            # iterations c=1..3: attention(pair c-1) woven into chunk c
            for c in range(1, NC4):
                B0, B1 = 2 * (c - 1), 2 * (c - 1) + 1
                C0, C1 = 2 * c, 2 * c + 1
                qkt = [(p, n) for p in ("q", "k") for n in range(ND)]

                sc_unit(B0, 0)
                sc_unit(B0, 1)
                qk_tile(c, *qkt[0])
                sc_unit(B0, 2); av_unit(B0, 0)
                qk_tile(c, *qkt[1])
                sc_unit(B0, 3); av_unit(B0, 1)
                qk_tile(c, *qkt[2])
                sc_unit(B0, 4); av_unit(B0, 2)
                qk_tile(c, *qkt[3])
                sc_unit(B0, 5); av_unit(B0, 3)
                qk_tile(c, *qkt[4])
                sc_unit(B1, 0); av_unit(B0, 4)
                qk_tile(c, *qkt[5])
                sc_unit(B1, 1); av_unit(B0, 5)
                recip_unit(B0)
                qk_tile(c, *qkt[6])
                sc_unit(B1, 2); av_unit(B1, 0)
                bc_unit(B0, 0); bc_unit(B0, 1)
                qk_tile(c, *qkt[7])
                sc_unit(B1, 3); av_unit(B1, 1)
                bc_unit(B0, 2); bc_unit(B0, 3)
                qk_tile(c, *qkt[8])
                sc_unit(B1, 4); av_unit(B1, 2)
                bc_unit(B0, 4); bc_unit(B0, 5)
                qk_tile(c, *qkt[9])
                sc_unit(B1, 5); av_unit(B1, 3)
                op_unit(B0, 0)
                qk_tile(c, *qkt[10])
                av_unit(B1, 4)
                op_unit(B0, 1)
                qk_tile(c, *qkt[11])
                av_unit(B1, 5)
                recip_unit(B1)
                vp_unit(C0, 0); vp_unit(C0, 1)
                bc_unit(B1, 0); bc_unit(B1, 1); bc_unit(B1, 2)
                vp_unit(C0, 2); vp_unit(C0, 3)
                bc_unit(B1, 3); bc_unit(B1, 4); bc_unit(B1, 5)
                vp_unit(C1, 0); vp_unit(C1, 1)
                op_unit(B1, 0)
                vp_unit(C1, 2); vp_unit(C1, 3)
                op_unit(B1, 1)

            # tail: attention for pair 3 (no projection work left to weave)
            B0, B1 = 6, 7
            sc_unit(B0, 0); sc_unit(B0, 1); sc_unit(B0, 2)
            sc_unit(B0, 3); av_unit(B0, 0)
            sc_unit(B0, 4); av_unit(B0, 1)
            sc_unit(B0, 5); av_unit(B0, 2)
            sc_unit(B1, 0); av_unit(B0, 3)
            sc_unit(B1, 1); av_unit(B0, 4)
            sc_unit(B1, 2); av_unit(B0, 5)
            recip_unit(B0)
            sc_unit(B1, 3); av_unit(B1, 0)
            bc_unit(B0, 0); bc_unit(B0, 1)
            sc_unit(B1, 4); av_unit(B1, 1)
            bc_unit(B0, 2); bc_unit(B0, 3)
            sc_unit(B1, 5); av_unit(B1, 2)
            bc_unit(B0, 4); bc_unit(B0, 5)
            av_unit(B1, 3)
            op_unit(B0, 0)
            av_unit(B1, 4)
            op_unit(B0, 1)
            av_unit(B1, 5)
            recip_unit(B1)
            for hp in range(ND):
                bc_unit(B1, hp)
            op_unit(B1, 0)
            op_unit(B1, 1)

    nc.compile()
    return nc


def _split16(a):
    hi = a.astype(np.float16)
    lo = (a - hi.astype(np.float32)).astype(np.float16)
    return hi, lo


def _prep_weights(Wq, bq, Wk, bk, Wv, bv, Wo, bo, has_bias):
    f32 = np.float32
    wq = np.asarray(Wq, f32) * f32(0.125)
    wk = np.asarray(Wk, f32)
    wq_hi, wq_lo = _split16(wq)
    wk_hi, wk_lo = _split16(wk)
    w = {
        "wq_hi": wq_hi, "wq_lo": wq_lo,
        "wk_hi": wk_hi, "wk_lo": wk_lo,
        "wv": np.asarray(Wv, f32).astype(np.float16),
        "wo": np.asarray(Wo, f32).astype(np.float16),
    }
    if has_bias:
        w["bq"] = (np.asarray(bq, f32) * f32(0.125)).astype(
            np.float16).reshape(1, D)
        w["bk"] = np.asarray(bk, f32).astype(np.float16).reshape(1, D)
        w["bv"] = np.asarray(bv, f32).astype(np.float16).reshape(1, D)
        w["bo"] = np.asarray(bo, f32).astype(np.float16).reshape(1, D)
    return w


def _make_in_maps(x, w):
    x = np.asarray(x, np.float32)
    in_maps = []
    for c in range(NCORES):
        m = dict(w)
        xc = x[c * BL:(c + 1) * BL].reshape(R, D)
        x16 = np.zeros((RPAD, D), np.float16)
        x16[:R] = xc.astype(np.float16)
        m["x16"] = x16
        if N_TERMS == 3:
            xlo = np.zeros((RPAD, D), np.float16)
            xlo[:R] = (xc - x16[:R].astype(np.float32)).astype(np.float16)
            m["xlo"] = xlo
        in_maps.append(m)
    return in_maps


def kernel(x, Wq, bq, Wk, bk, Wv, bv, Wo, bo):
    from concourse import bass_utils

    has_bias = any(float(np.abs(np.asarray(v)).max()) != 0.0
                   for v in (bq, bk, bv, bo))
    key = ("nc", has_bias, N_TERMS)
    if key not in _CACHE:
        _CACHE[key] = _build(has_bias, N_TERMS)
    nc = _CACHE[key]

    w = _prep_weights(Wq, bq, Wk, bk, Wv, bv, Wo, bo, has_bias)
    in_maps = _make_in_maps(x, w)

    res = bass_utils.run_bass_kernel_spmd(nc, in_maps, list(range(NCORES)))
    out = np.concatenate(
        [res.results[c]["out"].reshape(BL, T, D) for c in range(NCORES)],
        axis=0)
    return out.astype(np.float32)


# revision 3
# speedup vs baseline: 1.5784x; 1.0815x over previous
"""Multi-head attention forward on 8 TRN2 NeuronCores (data-parallel over batch).

Reference computation (B=64, T=197, D=768, H=12, DK=64, fp32):
    q = split_heads(x @ Wq + bq); k = ...; v = ...
    scores = floor((q @ k^T) / 8); attn = softmax(scores); out = attn @ v
    return merge_heads(out) @ Wo + bo

Numerics: floor() before softmax makes the Q/K path sensitive.  q/k
projections run as 2-term fp16 matmuls: W is split hi+lo (22-bit
mantissa), x is truncated to fp16 (the x_lo term is dropped):
q = x16 @ W_hi + x16 @ W_lo, exact fp32 PSUM accumulation.  Measured
rel err vs the fp32 reference: 1.5e-2 (budget 2e-2); the 3-term
variant (adds x_lo @ W_hi, rel err 1.2e-3) is kept behind N_TERMS=3.
The scores matmul is native fp32, two heads row-packed via
tile_position (packed pairs execute concurrently on the PE).  The V
path (v proj, attn@v, out proj) runs in plain fp16.

Layout: x is transposed + fp16-cast + partition-packed on the HOST
(x16p[p, k*1584+j] = x16[j, k*128+p]) and weights partition-packed
(w[p, k*768+c] = W[k*128+p, c]) so every DMA descriptor is a 9-19KB
contiguous run — the on-chip transpose phase and its 256B-descriptor
DMA-transpose storm are gone entirely (input DMA ~7us, was ~50us).

Schedule: one software-pipelined instruction stream so the PE never
idles (idle >3.4us re-throttles the PE clock to 1.2GHz via HAM).
Projections run in 4 column chunks of 394 rows = one batch pair each.
Attention for pair p (scores -> floor (DVE magic-number round) -> Exp
(ScalarE, -MAGIC folded into the bias) -> attn@v -> normalize -> out
proj) is woven into projection chunk p+1's matmul stream; v-proj units
(dependency-free) pad the latency-sensitive spots (reciprocal ->
broadcast).  Pair 3's scores are pulled into iteration 3 so the tail
is short.  attn@v right-appends a ones column per head (v stride 65)
so the softmax denominator falls out of the same matmul; denominators
are gathered, one batched reciprocal, PE-broadcast, DVE multiply.  PE
warmup matmuls + Exp-table preload run during the initial DMA wait.

All PSUM tiles come from one shared-tag pool (8 banks round-robin).
"""

import numpy as np

B, T, D, H, DK = 64, 197, 768, 12, 64
NCORES = 8
BL = B // NCORES          # 8 batch elements per core
R = BL * T                # 1576 rows per core
RPAD = 1584               # row count padded (keeps host packing regular)
ND = D // 128             # 6 chunks of 128 along D
NC4 = 4                   # proj col chunks (one batch pair each)
CW = R // NC4             # 394 = 2*T
HV = DK + 1               # 65: per-head v stride (ones column at 64)
KEYCHUNKS = [(0, 128), (128, 69)]
MAGIC = float(3 * 2 ** 22)  # 1.5*2^23: x-0.5+MAGIC stays in [2^23,2^24), ulp=1
N_TERMS = 2               # 2: q/k = x16@W_hi + x16@W_lo; 3: + xlo@W_hi

_CACHE = {}


def _build(has_bias, n_terms):
    import concourse.bacc as bacc
    import concourse.mybir as mybir
    import concourse.tile as tile

    f32 = mybir.dt.float32
    f16 = mybir.dt.float16
    AF = mybir.ActivationFunctionType
    OP = mybir.AluOpType

    nc = bacc.Bacc("TRN2", target_bir_lowering=False, debug=False,
                   num_devices=NCORES)

    x16_d = nc.dram_tensor("x16p", [128, ND * RPAD], f16,
                           kind="ExternalInput").ap()
    if n_terms == 3:
        xlo_d = nc.dram_tensor("xlop", [128, ND * RPAD], f16,
                               kind="ExternalInput").ap()
    w_d = {}
    for nm in ("wq_hi", "wq_lo", "wk_hi", "wk_lo", "wv", "wo"):
        w_d[nm] = nc.dram_tensor(nm, [128, ND * D], f16,
                                 kind="ExternalInput").ap()
    if has_bias:
        b_d = {nm: nc.dram_tensor(nm, [1, D], f16, kind="ExternalInput").ap()
               for nm in ("bq", "bk", "bv", "bo")}
    out_d = nc.dram_tensor("out", [R, D], f32, kind="ExternalOutput").ap()

    with tile.TileContext(nc) as tc:
        with tc.tile_pool(name="static", bufs=1) as Ps, \
             tc.tile_pool(name="work", bufs=1) as Pw, \
             tc.tile_pool(name="psum", bufs=8, space="PSUM") as Pp:

            def ptile(nm):
                return Pp.tile([128, CW], f32, name=nm, tag="ps", bufs=8,
                               uniquify=True)

            xall = Ps.tile([128, ND * RPAD], f16, name="xall")
            if n_terms == 3:
                xloall = Ps.tile([128, ND * RPAD], f16, name="xloall")
            wsb = {nm: Ps.tile([128, ND * D], f16, name=nm)
                   for nm in ("wq_hi", "wq_lo", "wk_hi", "wk_lo", "wv", "wo")}
            # v16e[2b+kc][keys<=128, 12*65]; col h*65+64 holds ones
            v16e = [Ps.tile([128, H * HV], f16, name=f"v16e_{i}")
                    for i in range(2 * BL)]
            ones_row = Ps.tile([128, CW], f16, name="ones_row")
            negmagic = Ps.tile([128, 1], f32, name="negmagic")
            prime = Ps.tile([1, 1], f16, name="prime")
            if has_bias:
                bsb = {nm: Ps.tile([1, D], f16, name=f"{nm}_sb")
                       for nm in ("bq", "bk", "bv", "bo")}

            def xs(k, c0, ln):
                return xall[:, k * RPAD + c0:k * RPAD + c0 + ln]

            def xls(k, c0, ln):
                return xloall[:, k * RPAD + c0:k * RPAD + c0 + ln]

            def ws(nm, k, c0, ln):
                return wsb[nm][:, k * D + c0:k * D + c0 + ln]

            # ---- no-DMA-dependency setup: memsets, engine warmups ----
            nc.vector.memset(ones_row, 1.0)
            nc.vector.memset(negmagic, -MAGIC)
            for i in range(2 * BL):
                onescol = v16e[i].rearrange("p (h c) -> p h c",
                                            c=HV)[:, :, DK:DK + 1]
                nc.gpsimd.memset(onescol, 1.0)
            # Exp table preload on ScalarE (one-time 1.3us table load)
            nc.scalar.activation(prime, ones_row[:1, :1], AF.Exp,
                                 bias=negmagic[:1, :1])
            # PE warmup: keep HAM at full clock until real work arrives
            for i in range(12):
                wu = ptile("wu")
                nc.tensor.matmul(wu, ones_row[:, :128], ones_row,
                                 start=True, stop=True)

            # ---- DMAs (all large contiguous descriptors) ----
            nc.sync.dma_start(wsb["wq_hi"], w_d["wq_hi"])
            nc.sync.dma_start(xall, x16_d)
            nc.sync.dma_start(wsb["wq_lo"], w_d["wq_lo"])
            if n_terms == 3:
                nc.sync.dma_start(xloall, xlo_d)
            for nm in ("wk_hi", "wk_lo", "wv", "wo"):
                nc.sync.dma_start(wsb[nm], w_d[nm])
            if has_bias:
                for nm in ("bq", "bk", "bv", "bo"):
                    nc.sync.dma_start(bsb[nm], b_d[nm])

            # ---- stage helpers (each call ISSUES instructions) ----
            qT = {}   # (proj, c, n) -> sbuf tile [128, CW] f32
            eTs = {}  # (b, hp) -> [e_t hl0, e_t hl1]
            otfs = {}  # (b, hp) -> otf tile
            oT16s = {}  # (b, hp) -> oT16 tile
            dns = {}
            rd16s = {}

            def qk_tile(c, proj, n):
                whi, wlo, b_nm = (("wq_hi", "wq_lo", "bq") if proj == "q"
                                  else ("wk_hi", "wk_lo", "bk"))
                c0 = c * CW
                pp = ptile("pp")
                for k in range(ND):
                    nc.tensor.matmul(pp, ws(whi, k, n * 128, 128),
                                     xs(k, c0, CW), start=(k == 0),
                                     stop=False)
                for k in range(ND):
                    last = (k == ND - 1 and n_terms == 2 and not has_bias)
                    nc.tensor.matmul(pp, ws(wlo, k, n * 128, 128),
                                     xs(k, c0, CW), start=False, stop=last)
                if n_terms == 3:
                    for k in range(ND):
                        last = (k == ND - 1 and not has_bias)
                        nc.tensor.matmul(pp, ws(whi, k, n * 128, 128),
                                         xls(k, c0, CW),
                                         start=False, stop=last)
                if has_bias:
                    nc.tensor.matmul(pp, bsb[b_nm][:1, n * 128:n * 128 + 128],
                                     ones_row[:1, :CW],
                                     start=False, stop=True)
                dst = Pw.tile([128, CW], f32, name=f"{proj}T", tag=f"{proj}T",
                              bufs=12, uniquify=True)
                nc.scalar.activation(dst, pp, AF.Copy)
                qT[(proj, c, n)] = dst

            def vp_unit(b, j):
                kc, half = j // 2, j % 2
                koff, klen = KEYCHUNKS[kc]
                base = b * T
                c0 = half * 384
                vp = ptile("vp")
                vps = vp[:klen, :384]
                for d in range(ND):
                    nc.tensor.matmul(
                        vps, xs(d, base + koff, klen), ws("wv", d, c0, 384),
                        start=(d == 0),
                        stop=(d == ND - 1 and not has_bias))
                if has_bias:
                    nc.tensor.matmul(vps, ones_row[:1, :klen],
                                     bsb["bv"][:1, c0:c0 + 384],
                                     start=False, stop=True)
                dst = v16e[2 * b + kc]
                dst3 = dst[:klen, :].rearrange("p (h c) -> p h c",
                                               c=HV)[:, :, 0:DK]
                nc.scalar.activation(
                    dst3[:, half * 6:(half + 1) * 6, :],
                    vps.rearrange("p (h c) -> p h c", c=DK), AF.Copy)

            def sc_unit(b, hp):
                c = b // 2
                qoff = (b % 2) * T
                eT = []
                for hl in range(2):
                    pb = 64 * hl
                    sc = ptile("sc")
                    for kc, (koff, klen) in enumerate(KEYCHUNKS):
                        nc.tensor.matmul(
                            sc[:klen, kc * T:(kc + 1) * T],
                            qT[("k", c, hp)][pb:pb + 64,
                                             qoff + koff:qoff + koff + klen],
                            qT[("q", c, hp)][pb:pb + 64, qoff:qoff + T],
                            start=True, stop=True, tile_position=(pb, 0))
                    fl = Pw.tile([128, 2 * T], f32, name="fl", tag="fl",
                                 bufs=6, uniquify=True)
                    nc.vector.tensor_scalar(fl, sc, -0.5, MAGIC,
                                            OP.add, OP.add)
                    e_t = Pw.tile([128, 2 * T], f16, name="e_t", tag="eT",
                                  bufs=16, uniquify=True)
                    nc.scalar.activation(e_t, fl, AF.Exp,
                                         bias=negmagic[:, :1])
                    eT.append(e_t)
                eTs[(b, hp)] = eT

            def av_unit(b, hp):
                eT = eTs.pop((b, hp))
                if hp == 0:
                    dns[b] = Pw.tile([128, 3 * T], f32, name="dn", tag="dn",
                                     bufs=3, uniquify=True)
                dn = dns[b]
                otf = Pw.tile([128, T], f32, name="otf", tag="otf",
                              bufs=14, uniquify=True)
                op_ = ptile("oT")
                for hl in range(2):
                    h = 2 * hp + hl
                    for kc, (koff, klen) in enumerate(KEYCHUNKS):
                        nc.tensor.matmul(
                            op_[0:HV, hl * T:(hl + 1) * T],
                            v16e[2 * b + kc][:klen, h * HV:(h + 1) * HV],
                            eT[hl][:klen, kc * T:(kc + 1) * T],
                            start=(kc == 0), stop=(kc == len(KEYCHUNKS) - 1))
                    pbase = 32 * (h % 4)
                    cb = (h // 4) * T
                    nc.vector.tensor_copy(dn[pbase:pbase + 1, cb:cb + T],
                                          op_[64:65, hl * T:(hl + 1) * T])
                    if hl == 0:
                        nc.scalar.activation(otf[0:64, :], op_[0:64, :T],
                                             AF.Copy)
                    else:
                        nc.vector.tensor_copy(otf[64:128, :],
                                              op_[0:64, T:2 * T])
                otfs[(b, hp)] = otf

            def recip_unit(b):
                dn = dns.pop(b)
                rdf = Pw.tile([128, 3 * T], f32, name="rdf", tag="rdf",
                              bufs=2, uniquify=True)
                rd16 = Pw.tile([128, 3 * T], f16, name="rd16", tag="rd16",
                               bufs=3, uniquify=True)
                nc.vector.reciprocal_approx_fast(rdf, dn)
                nc.vector.tensor_copy(rd16, rdf)
                rd16s[b] = rd16

            def bc_unit(b, hp):
                rd16 = rd16s[b]
                bc = ptile("bc")
                for hl in range(2):
                    h = 2 * hp + hl
                    pbase = 32 * (h % 4)
                    cb = (h // 4) * T
                    nc.tensor.matmul(
                        bc[64 * hl:64 * hl + 64, :T],
                        ones_row[pbase:pbase + 1, :64],
                        rd16[pbase:pbase + 1, cb:cb + T],
                        start=True, stop=True,
                        tile_position=(pbase, 64 * hl))
                oT16 = Pw.tile([128, T], f16, name="oT16", tag="oT16",
                               bufs=14, uniquify=True)
                nc.vector.tensor_tensor(oT16, otfs.pop((b, hp)), bc[:, :T],
                                        OP.mult)
                oT16s[(b, hp)] = oT16

            def op_unit(b, rc):
                base = b * T
                roff, rn = (0, 128) if rc == 0 else (128, T - 128)
                fs = Pw.tile([128, D], f32, name="fs", tag="fs",
                             bufs=4, uniquify=True)
                for half in range(2):
                    c0 = half * 384
                    fp_ = ptile("fp")
                    for d in range(ND):
                        nc.tensor.matmul(
                            fp_[:rn, :384],
                            oT16s[(b, d)][:, roff:roff + rn],
                            ws("wo", d, c0, 384),
                            start=(d == 0),
                            stop=(d == ND - 1 and not has_bias))
                    if has_bias:
                        nc.tensor.matmul(fp_[:rn, :384], ones_row[:1, :rn],
                                         bsb["bo"][:1, c0:c0 + 384],
                                         start=False, stop=True)
                    nc.scalar.activation(fs[:rn, c0:c0 + 384],
                                         fp_[:rn, :384], AF.Copy)
                nc.sync.dma_start(out_d[base + roff:base + roff + rn, :],
                                  fs[:rn, :])
                if rc == 1:
                    for d in range(ND):
                        del oT16s[(b, d)]

            # ---- pipelined program ----
            # c0 prologue: projections chunk 0 + v for pair 0
            for n in range(ND):
                qk_tile(0, "q", n)
            for n in range(ND):
                qk_tile(0, "k", n)
            for j in range(4):
                vp_unit(0, j)
            for j in range(4):
                vp_unit(1, j)

            # iterations c=1..3: attention(pair c-1) woven into chunk c
            for c in range(1, NC4):
                B0, B1 = 2 * (c - 1), 2 * (c - 1) + 1
                C0, C1 = 2 * c, 2 * c + 1
                qkt = [(p, n) for p in ("q", "k") for n in range(ND)]

                sc_unit(B0, 0); sc_unit(B0, 1)
                qk_tile(c, *qkt[0])
                sc_unit(B0, 2); av_unit(B0, 0)
                qk_tile(c, *qkt[1])
                sc_unit(B0, 3); av_unit(B0, 1)
                qk_tile(c, *qkt[2])
                sc_unit(B0, 4); av_unit(B0, 2)
                qk_tile(c, *qkt[3])
                sc_unit(B0, 5); av_unit(B0, 3)
                qk_tile(c, *qkt[4])
                av_unit(B0, 4); av_unit(B0, 5)
                recip_unit(B0)
                qk_tile(c, *qkt[5])
                sc_unit(B1, 0); sc_unit(B1, 1)
                qk_tile(c, *qkt[6])
                sc_unit(B1, 2); av_unit(B1, 0)
                qk_tile(c, *qkt[7])
                sc_unit(B1, 3); av_unit(B1, 1)
                bc_unit(B0, 0); bc_unit(B0, 1); bc_unit(B0, 2)
                qk_tile(c, *qkt[8])
                sc_unit(B1, 4); av_unit(B1, 2)
                bc_unit(B0, 3); bc_unit(B0, 4); bc_unit(B0, 5)
                qk_tile(c, *qkt[9])
                sc_unit(B1, 5); av_unit(B1, 3)
                op_unit(B0, 0)
                qk_tile(c, *qkt[10])
                av_unit(B1, 4)
                op_unit(B0, 1)
                qk_tile(c, *qkt[11])
                av_unit(B1, 5)
                recip_unit(B1)
                if c < 3:
                    vp_unit(C0, 0); vp_unit(C0, 1)
                    bc_unit(B1, 0); bc_unit(B1, 1); bc_unit(B1, 2)
                    vp_unit(C0, 2); vp_unit(C0, 3)
                    bc_unit(B1, 3); bc_unit(B1, 4); bc_unit(B1, 5)
                    vp_unit(C1, 0); vp_unit(C1, 1)
                    op_unit(B1, 0)
                    vp_unit(C1, 2); vp_unit(C1, 3)
                    op_unit(B1, 1)
                else:
                    # extended weave: pair-3 scores start here (chunk 3 done)
                    sc_unit(6, 0); sc_unit(6, 1)
                    vp_unit(6, 0); vp_unit(6, 1)
                    bc_unit(5, 0); bc_unit(5, 1); bc_unit(5, 2)
                    sc_unit(6, 2); sc_unit(6, 3)
                    vp_unit(6, 2); vp_unit(6, 3)
                    bc_unit(5, 3); bc_unit(5, 4); bc_unit(5, 5)
                    sc_unit(6, 4); sc_unit(6, 5)
                    vp_unit(7, 0); vp_unit(7, 1)
                    op_unit(5, 0)
                    sc_unit(7, 0); av_unit(6, 0)
                    vp_unit(7, 2); vp_unit(7, 3)
                    op_unit(5, 1)
                    sc_unit(7, 1); av_unit(6, 1)
                    sc_unit(7, 2); av_unit(6, 2)
                    sc_unit(7, 3); av_unit(6, 3)
                    sc_unit(7, 4); av_unit(6, 4)
                    sc_unit(7, 5); av_unit(6, 5)
                    recip_unit(6)
                    av_unit(7, 0); av_unit(7, 1)
                    bc_unit(6, 0); bc_unit(6, 1); bc_unit(6, 2)
                    av_unit(7, 2); av_unit(7, 3)
                    bc_unit(6, 3); bc_unit(6, 4); bc_unit(6, 5)
                    av_unit(7, 4); av_unit(7, 5)
                    recip_unit(7)
                    op_unit(6, 0)
                    bc_unit(7, 0); bc_unit(7, 1); bc_unit(7, 2)
                    op_unit(6, 1)
                    bc_unit(7, 3); bc_unit(7, 4); bc_unit(7, 5)
                    op_unit(7, 0)
                    op_unit(7, 1)

    nc.compile()
    return nc


def _split16(a):
    hi = a.astype(np.float16)
    lo = (a - hi.astype(np.float32)).astype(np.float16)
    return hi, lo


def _pack_w(a16):
    # [768, N] fp16 -> [128, 6*N]: out[p, k*N+c] = a16[k*128+p, c]
    N = a16.shape[1]
    return np.ascontiguousarray(
        a16.reshape(ND, 128, N).transpose(1, 0, 2).reshape(128, ND * N))


def _prep_weights(Wq, bq, Wk, bk, Wv, bv, Wo, bo, has_bias):
    f32 = np.float32
    wq = np.asarray(Wq, f32) * f32(0.125)
    wk = np.asarray(Wk, f32)
    wq_hi, wq_lo = _split16(wq)
    wk_hi, wk_lo = _split16(wk)
    w = {
        "wq_hi": _pack_w(wq_hi), "wq_lo": _pack_w(wq_lo),
        "wk_hi": _pack_w(wk_hi), "wk_lo": _pack_w(wk_lo),
        "wv": _pack_w(np.asarray(Wv, f32).astype(np.float16)),
        "wo": _pack_w(np.asarray(Wo, f32).astype(np.float16)),
    }
    if has_bias:
        w["bq"] = (np.asarray(bq, f32) * f32(0.125)).astype(
            np.float16).reshape(1, D)
        w["bk"] = np.asarray(bk, f32).astype(np.float16).reshape(1, D)
        w["bv"] = np.asarray(bv, f32).astype(np.float16).reshape(1, D)
        w["bo"] = np.asarray(bo, f32).astype(np.float16).reshape(1, D)
    return w


def _pack_x(xc16):
    # [R, 768] fp16 -> [128, 6*RPAD]: out[p, k*RPAD+j] = x16[j, k*128+p]
    xt = np.zeros((D, RPAD), np.float16)
    xt[:, :R] = xc16.T
    return np.ascontiguousarray(
        xt.reshape(ND, 128, RPAD).transpose(1, 0, 2).reshape(128, ND * RPAD))


def _make_in_maps(x, w):
    x = np.asarray(x, np.float32)
    in_maps = []
    for c in range(NCORES):
        m = dict(w)
        xc = x[c * BL:(c + 1) * BL].reshape(R, D)
        x16 = xc.astype(np.float16)
        m["x16p"] = _pack_x(x16)
        if N_TERMS == 3:
            m["xlop"] = _pack_x(
                (xc - x16.astype(np.float32)).astype(np.float16))
        in_maps.append(m)
    return in_maps


def kernel(x, Wq, bq, Wk, bk, Wv, bv, Wo, bo):
    from concourse import bass_utils

    has_bias = any(float(np.abs(np.asarray(v)).max()) != 0.0
                   for v in (bq, bk, bv, bo))
    key = ("nc", has_bias, N_TERMS)
    if key not in _CACHE:
        _CACHE[key] = _build(has_bias, N_TERMS)
    nc = _CACHE[key]

    w = _prep_weights(Wq, bq, Wk, bk, Wv, bv, Wo, bo, has_bias)
    in_maps = _make_in_maps(x, w)

    res = bass_utils.run_bass_kernel_spmd(nc, in_maps, list(range(NCORES)))
    out = np.concatenate(
        [res.results[c]["out"].reshape(BL, T, D) for c in range(NCORES)],
        axis=0)
    return out.astype(np.float32)


# revision 7
# speedup vs baseline: 1.6100x; 1.0201x over previous
"""Multi-head attention forward on 8 TRN2 NeuronCores (data-parallel over batch).

Reference computation (B=64, T=197, D=768, H=12, DK=64, fp32):
    q = split_heads(x @ Wq + bq); k = ...; v = ...
    scores = floor((q @ k^T) / 8); attn = softmax(scores); out = attn @ v
    return merge_heads(out) @ Wo + bo

Numerics: floor() before softmax makes the Q/K path sensitive.  q/k
projections run as 2-term fp16 matmuls: W is split hi+lo (22-bit
mantissa), x is truncated to fp16 (the x_lo term is dropped):
q = x16 @ W_hi + x16 @ W_lo, exact fp32 PSUM accumulation.  Measured
rel err vs the fp32 reference: 1.5e-2 (budget 2e-2); the 3-term
variant (adds x_lo @ W_hi, rel err 1.2e-3) is kept behind N_TERMS=3.
The scores matmul is native fp32, two heads row-packed via
tile_position (packed pairs execute concurrently on the PE).  The V
path (v proj, attn@v, out proj) runs in plain fp16.

Layout: x is transposed + fp16-cast + partition-packed on the HOST
(x16p[p, k*1584+j] = x16[j, k*128+p]) and weights partition-packed
(w[p, k*768+c] = W[k*128+p, c]) so every DMA descriptor is a 9-19KB
contiguous run — the on-chip transpose phase and its 256B-descriptor
DMA-transpose storm are gone entirely (input DMA ~7us, was ~50us).

Schedule: one software-pipelined instruction stream so the PE never
idles (idle >3.4us re-throttles the PE clock to 1.2GHz via HAM).
Projections run in 4 column chunks of 394 rows = one batch pair each.
Attention for pair p (scores -> floor (DVE magic-number round) -> Exp
(ScalarE, -MAGIC folded into the bias) -> attn@v -> normalize -> out
proj) is woven into projection chunk p+1's matmul stream; v-proj units
(dependency-free) pad the latency-sensitive spots (reciprocal ->
broadcast).  Pair 3's scores are pulled into iteration 3 so the tail
is short.  attn@v right-appends a ones column per head (v stride 65)
so the softmax denominator falls out of the same matmul; denominators
are gathered, one batched reciprocal, PE-broadcast, DVE multiply.  PE
warmup matmuls + Exp-table preload run during the initial DMA wait.

All PSUM tiles come from one shared-tag pool (8 banks round-robin).
"""

import numpy as np

B, T, D, H, DK = 64, 197, 768, 12, 64
NCORES = 8
BL = B // NCORES          # 8 batch elements per core
R = BL * T                # 1576 rows per core
RPAD = 1584               # row count padded (keeps host packing regular)
ND = D // 128             # 6 chunks of 128 along D
NC4 = 4                   # proj col chunks (one batch pair each)
CW = R // NC4             # 394 = 2*T
HV = DK + 1               # 65: per-head v stride (ones column at 64)
KEYCHUNKS = [(0, 128), (128, 69)]
MAGIC = float(3 * 2 ** 22)  # 1.5*2^23: x-0.5+MAGIC stays in [2^23,2^24), ulp=1
N_TERMS = 2               # 2: q/k = x16@W_hi + x16@W_lo; 3: + xlo@W_hi

_CACHE = {}


def _build(has_bias, n_terms):
    import concourse.bacc as bacc
    import concourse.mybir as mybir
    import concourse.tile as tile

    f32 = mybir.dt.float32
    f16 = mybir.dt.float16
    AF = mybir.ActivationFunctionType
    OP = mybir.AluOpType

    nc = bacc.Bacc("TRN2", target_bir_lowering=False, debug=False,
                   num_devices=NCORES)

    x16_d = nc.dram_tensor("x16p", [128, ND * RPAD], f16,
                           kind="ExternalInput").ap()
    if n_terms == 3:
        xlo_d = nc.dram_tensor("xlop", [128, ND * RPAD], f16,
                               kind="ExternalInput").ap()
    w_d = {}
    for nm in ("wq_hi", "wq_lo", "wk_hi", "wk_lo", "wv", "wo"):
        w_d[nm] = nc.dram_tensor(nm, [128, ND * D], f16,
                                 kind="ExternalInput").ap()
    if has_bias:
        b_d = {nm: nc.dram_tensor(nm, [1, D], f16, kind="ExternalInput").ap()
               for nm in ("bq", "bk", "bv", "bo")}
    out_d = nc.dram_tensor("out", [R, D], f32, kind="ExternalOutput").ap()

    with tile.TileContext(nc) as tc:
        with tc.tile_pool(name="static", bufs=1) as Ps, \
             tc.tile_pool(name="work", bufs=1) as Pw, \
             tc.tile_pool(name="psum", bufs=8, space="PSUM") as Pp:

            def ptile(nm):
                return Pp.tile([128, CW], f32, name=nm, tag="ps", bufs=8,
                               uniquify=True)

            xall = Ps.tile([128, ND * RPAD], f16, name="xall")
            if n_terms == 3:
                xloall = Ps.tile([128, ND * RPAD], f16, name="xloall")
            wsb = {nm: Ps.tile([128, ND * D], f16, name=nm)
                   for nm in ("wq_hi", "wq_lo", "wk_hi", "wk_lo", "wv", "wo")}
            # v16e[2b+kc][keys<=128, 12*65]; col h*65+64 holds ones
            v16e = [Ps.tile([128, H * HV], f16, name=f"v16e_{i}")
                    for i in range(2 * BL)]
            ones_row = Ps.tile([128, CW], f16, name="ones_row")
            negmagic = Ps.tile([128, 1], f32, name="negmagic")
            prime = Ps.tile([1, 1], f16, name="prime")
            if has_bias:
                bsb = {nm: Ps.tile([1, D], f16, name=f"{nm}_sb")
                       for nm in ("bq", "bk", "bv", "bo")}

            def xs(k, c0, ln):
                return xall[:, k * RPAD + c0:k * RPAD + c0 + ln]

            def xls(k, c0, ln):
                return xloall[:, k * RPAD + c0:k * RPAD + c0 + ln]

            def ws(nm, k, c0, ln):
                return wsb[nm][:, k * D + c0:k * D + c0 + ln]

            # ---- no-DMA-dependency setup: memsets, engine warmups ----
            nc.vector.memset(ones_row, 1.0)
            nc.vector.memset(negmagic, -MAGIC)
            for i in range(2 * BL):
                onescol = v16e[i].rearrange("p (h c) -> p h c",
                                            c=HV)[:, :, DK:DK + 1]
                nc.gpsimd.memset(onescol, 1.0)
            # Exp table preload on ScalarE (one-time 1.3us table load)
            nc.scalar.activation(prime, ones_row[:1, :1], AF.Exp,
                                 bias=negmagic[:1, :1])
            # PE warmup: keep HAM at full clock until real work arrives
            for i in range(34):
                wu = ptile("wu")
                nc.tensor.matmul(wu, ones_row[:, :128], ones_row,
                                 start=True, stop=True)

            # ---- DMAs (all large contiguous descriptors) ----
            nc.sync.dma_start(wsb["wq_hi"], w_d["wq_hi"])
            nc.sync.dma_start(xall, x16_d)
            nc.sync.dma_start(wsb["wq_lo"], w_d["wq_lo"])
            if n_terms == 3:
                nc.sync.dma_start(xloall, xlo_d)
            for nm in ("wk_hi", "wk_lo", "wv", "wo"):
                nc.sync.dma_start(wsb[nm], w_d[nm])
            if has_bias:
                for nm in ("bq", "bk", "bv", "bo"):
                    nc.sync.dma_start(bsb[nm], b_d[nm])

            # ---- stage helpers (each call ISSUES instructions) ----
            qT = {}   # (proj, c, n) -> sbuf tile [128, CW] f32
            eTs = {}  # (b, hp) -> [e_t hl0, e_t hl1]
            otfs = {}  # (b, hp) -> otf tile
            oT16s = {}  # (b, hp) -> oT16 tile
            dns = {}
            rd16s = {}

            def qk_tile(c, proj, n):
                whi, wlo, b_nm = (("wq_hi", "wq_lo", "bq") if proj == "q"
                                  else ("wk_hi", "wk_lo", "bk"))
                c0 = c * CW
                pp = ptile("pp")
                for k in range(ND):
                    nc.tensor.matmul(pp, ws(whi, k, n * 128, 128),
                                     xs(k, c0, CW), start=(k == 0),
                                     stop=False)
                for k in range(ND):
                    last = (k == ND - 1 and n_terms == 2 and not has_bias)
                    nc.tensor.matmul(pp, ws(wlo, k, n * 128, 128),
                                     xs(k, c0, CW), start=False, stop=last)
                if n_terms == 3:
                    for k in range(ND):
                        last = (k == ND - 1 and not has_bias)
                        nc.tensor.matmul(pp, ws(whi, k, n * 128, 128),
                                         xls(k, c0, CW),
                                         start=False, stop=last)
                if has_bias:
                    nc.tensor.matmul(pp, bsb[b_nm][:1, n * 128:n * 128 + 128],
                                     ones_row[:1, :CW],
                                     start=False, stop=True)
                dst = Pw.tile([128, CW], f32, name=f"{proj}T", tag=f"{proj}T",
                              bufs=12, uniquify=True)
                nc.scalar.activation(dst, pp, AF.Copy)
                qT[(proj, c, n)] = dst

            def vp_unit(b, j):
                kc, half = j // 2, j % 2
                koff, klen = KEYCHUNKS[kc]
                base = b * T
                c0 = half * 384
                vp = ptile("vp")
                vps = vp[:klen, :384]
                for d in range(ND):
                    nc.tensor.matmul(
                        vps, xs(d, base + koff, klen), ws("wv", d, c0, 384),
                        start=(d == 0),
                        stop=(d == ND - 1 and not has_bias))
                if has_bias:
                    nc.tensor.matmul(vps, ones_row[:1, :klen],
                                     bsb["bv"][:1, c0:c0 + 384],
                                     start=False, stop=True)
                dst = v16e[2 * b + kc]
                dst3 = dst[:klen, :].rearrange("p (h c) -> p h c",
                                               c=HV)[:, :, 0:DK]
                nc.scalar.activation(
                    dst3[:, half * 6:(half + 1) * 6, :],
                    vps.rearrange("p (h c) -> p h c", c=DK), AF.Copy)

            def sc_unit(b, hp):
                c = b // 2
                qoff = (b % 2) * T
                eT = []
                for hl in range(2):
                    pb = 64 * hl
                    sc = ptile("sc")
                    for kc, (koff, klen) in enumerate(KEYCHUNKS):
                        nc.tensor.matmul(
                            sc[:klen, kc * T:(kc + 1) * T],
                            qT[("k", c, hp)][pb:pb + 64,
                                             qoff + koff:qoff + koff + klen],
                            qT[("q", c, hp)][pb:pb + 64, qoff:qoff + T],
                            start=True, stop=True, tile_position=(pb, 0))
                    fl = Pw.tile([128, 2 * T], f32, name="fl", tag="fl",
                                 bufs=6, uniquify=True)
                    nc.vector.tensor_scalar(fl, sc, -0.5, MAGIC,
                                            OP.add, OP.add)
                    e_t = Pw.tile([128, 2 * T], f16, name="e_t", tag="eT",
                                  bufs=16, uniquify=True)
                    nc.scalar.activation(e_t, fl, AF.Exp,
                                         bias=negmagic[:, :1])
                    eT.append(e_t)
                eTs[(b, hp)] = eT

            def av_unit(b, hp):
                eT = eTs.pop((b, hp))
                if hp == 0:
                    # denominator gather split in two so the reciprocal can
                    # start after hp 0-3 (heads 0-7) instead of after all 12
                    dns[b] = (
                        Pw.tile([128, 2 * T], f32, name="dnA", tag="dnA",
                                bufs=3, uniquify=True),
                        Pw.tile([128, T], f32, name="dnB", tag="dnB",
                                bufs=3, uniquify=True))
                dnA, dnB = dns[b]
                otf = Pw.tile([128, T], f32, name="otf", tag="otf",
                              bufs=14, uniquify=True)
                op_ = ptile("oT")
                for hl in range(2):
                    h = 2 * hp + hl
                    for kc, (koff, klen) in enumerate(KEYCHUNKS):
                        nc.tensor.matmul(
                            op_[0:HV, hl * T:(hl + 1) * T],
                            v16e[2 * b + kc][:klen, h * HV:(h + 1) * HV],
                            eT[hl][:klen, kc * T:(kc + 1) * T],
                            start=(kc == 0), stop=(kc == len(KEYCHUNKS) - 1))
                    pbase = 32 * (h % 4)
                    dn, cb = (dnA, (h // 4) * T) if h < 8 else (dnB, 0)
                    nc.vector.tensor_copy(dn[pbase:pbase + 1, cb:cb + T],
                                          op_[64:65, hl * T:(hl + 1) * T])
                    if hl == 0:
                        nc.scalar.activation(otf[0:64, :], op_[0:64, :T],
                                             AF.Copy)
                    else:
                        nc.vector.tensor_copy(otf[64:128, :],
                                              op_[0:64, T:2 * T])
                otfs[(b, hp)] = otf

            def recip_unit(b, part):
                dnA, dnB = dns[b]
                if part == 0:
                    rdf = Pw.tile([128, 2 * T], f32, name="rdfA", tag="rdfA",
                                  bufs=2, uniquify=True)
                    rd16 = Pw.tile([128, 2 * T], f16, name="rdA", tag="rdA",
                                   bufs=3, uniquify=True)
                    nc.vector.reciprocal_approx_fast(rdf, dnA)
                    nc.gpsimd.tensor_copy(rd16, rdf)
                    rd16s[b] = [rd16, None]
                else:
                    rdf = Pw.tile([128, T], f32, name="rdfB", tag="rdfB",
                                  bufs=2, uniquify=True)
                    rd16 = Pw.tile([128, T], f16, name="rdB", tag="rdB",
                                   bufs=3, uniquify=True)
                    nc.vector.reciprocal_approx_fast(rdf, dnB)
                    nc.gpsimd.tensor_copy(rd16, rdf)
                    rd16s[b][1] = rd16
                    del dns[b]

            def bc_unit(b, hp):
                bc = ptile("bc")
                for hl in range(2):
                    h = 2 * hp + hl
                    pbase = 32 * (h % 4)
                    rd16, cb = ((rd16s[b][0], (h // 4) * T) if h < 8
                                else (rd16s[b][1], 0))
                    nc.tensor.matmul(
                        bc[64 * hl:64 * hl + 64, :T],
                        ones_row[pbase:pbase + 1, :64],
                        rd16[pbase:pbase + 1, cb:cb + T],
                        start=True, stop=True,
                        tile_position=(pbase, 64 * hl))
                oT16 = Pw.tile([128, T], f16, name="oT16", tag="oT16",
                               bufs=14, uniquify=True)
                nc.vector.tensor_tensor(oT16, otfs.pop((b, hp)), bc[:, :T],
                                        OP.mult)
                oT16s[(b, hp)] = oT16

            def op_unit(b, rc):
                base = b * T
                roff, rn = (0, 128) if rc == 0 else (128, T - 128)
                fs = Pw.tile([128, D], f32, name="fs", tag="fs",
                             bufs=4, uniquify=True)
                for half in range(2):
                    c0 = half * 384
                    fp_ = ptile("fp")
                    for d in range(ND):
                        nc.tensor.matmul(
                            fp_[:rn, :384],
                            oT16s[(b, d)][:, roff:roff + rn],
                            ws("wo", d, c0, 384),
                            start=(d == 0),
                            stop=(d == ND - 1 and not has_bias))
                    if has_bias:
                        nc.tensor.matmul(fp_[:rn, :384], ones_row[:1, :rn],
                                         bsb["bo"][:1, c0:c0 + 384],
                                         start=False, stop=True)
                    nc.scalar.activation(fs[:rn, c0:c0 + 384],
                                         fp_[:rn, :384], AF.Copy)
                    nc.sync.dma_start(
                        out_d[base + roff:base + roff + rn, c0:c0 + 384],
                        fs[:rn, c0:c0 + 384])
                if rc == 1:
                    for d in range(ND):
                        del oT16s[(b, d)]

            # ---- pipelined program ----
            # c0 prologue: projections chunk 0 + v for pair 0
            for n in range(ND):
                qk_tile(0, "q", n)
            for n in range(ND):
                qk_tile(0, "k", n)
            for j in range(4):
                vp_unit(0, j)
            for j in range(4):
                vp_unit(1, j)

            # iterations c=1..3: attention(pair c-1) woven into chunk c
            for c in range(1, NC4):
                B0, B1 = 2 * (c - 1), 2 * (c - 1) + 1
                C0, C1 = 2 * c, 2 * c + 1
                qkt = [(p, n) for p in ("q", "k") for n in range(ND)]

                sc_unit(B0, 0); sc_unit(B0, 1)
                qk_tile(c, *qkt[0])
                sc_unit(B0, 2); av_unit(B0, 0)
                qk_tile(c, *qkt[1])
                sc_unit(B0, 3); av_unit(B0, 1)
                qk_tile(c, *qkt[2])
                sc_unit(B0, 4); av_unit(B0, 2)
                qk_tile(c, *qkt[3])
                sc_unit(B0, 5); av_unit(B0, 3)
                recip_unit(B0, 0)
                qk_tile(c, *qkt[4])
                av_unit(B0, 4); av_unit(B0, 5)
                recip_unit(B0, 1)
                qk_tile(c, *qkt[5])
                sc_unit(B1, 0); sc_unit(B1, 1)
                qk_tile(c, *qkt[6])
                sc_unit(B1, 2); av_unit(B1, 0)
                bc_unit(B0, 0); bc_unit(B0, 1)
                qk_tile(c, *qkt[7])
                sc_unit(B1, 3); av_unit(B1, 1)
                bc_unit(B0, 2); bc_unit(B0, 3)
                qk_tile(c, *qkt[8])
                sc_unit(B1, 4); av_unit(B1, 2)
                bc_unit(B0, 4); bc_unit(B0, 5)
                qk_tile(c, *qkt[9])
                sc_unit(B1, 5); av_unit(B1, 3)
                recip_unit(B1, 0)
                op_unit(B0, 0)
                qk_tile(c, *qkt[10])
                av_unit(B1, 4)
                op_unit(B0, 1)
                qk_tile(c, *qkt[11])
                av_unit(B1, 5)
                recip_unit(B1, 1)
                if c < 3:
                    vp_unit(C0, 0); vp_unit(C0, 1)
                    bc_unit(B1, 0); bc_unit(B1, 1); bc_unit(B1, 2)
                    vp_unit(C0, 2); vp_unit(C0, 3)
                    bc_unit(B1, 3); bc_unit(B1, 4); bc_unit(B1, 5)
                    vp_unit(C1, 0); vp_unit(C1, 1)
                    op_unit(B1, 0)
                    vp_unit(C1, 2); vp_unit(C1, 3)
                    op_unit(B1, 1)
                else:
                    # extended weave: pair-3 scores start here (chunk 3 done)
                    sc_unit(6, 0); sc_unit(6, 1)
                    vp_unit(6, 0); vp_unit(6, 1)
                    bc_unit(5, 0); bc_unit(5, 1); bc_unit(5, 2)
                    sc_unit(6, 2); sc_unit(6, 3)
                    vp_unit(6, 2); vp_unit(6, 3)
                    bc_unit(5, 3); bc_unit(5, 4); bc_unit(5, 5)
                    sc_unit(6, 4); sc_unit(6, 5)
                    vp_unit(7, 0); vp_unit(7, 1)
                    op_unit(5, 0)
                    sc_unit(7, 0); av_unit(6, 0)
                    vp_unit(7, 2); vp_unit(7, 3)
                    op_unit(5, 1)
                    sc_unit(7, 1); av_unit(6, 1)
                    sc_unit(7, 2); av_unit(6, 2)
                    sc_unit(7, 3); av_unit(6, 3)
                    recip_unit(6, 0)
                    sc_unit(7, 4); av_unit(6, 4)
                    sc_unit(7, 5); av_unit(6, 5)
                    recip_unit(6, 1)
                    av_unit(7, 0); av_unit(7, 1)
                    bc_unit(6, 0); bc_unit(6, 1); bc_unit(6, 2)
                    av_unit(7, 2); av_unit(7, 3)
                    recip_unit(7, 0)
                    bc_unit(6, 3); bc_unit(6, 4); bc_unit(6, 5)
                    av_unit(7, 4); av_unit(7, 5)
                    recip_unit(7, 1)
                    op_unit(6, 0)
                    bc_unit(7, 0); bc_unit(7, 1); bc_unit(7, 2)
                    bc_unit(7, 3)
                    op_unit(6, 1)
                    bc_unit(7, 4); bc_unit(7, 5)
                    op_unit(7, 0)
                    op_unit(7, 1)

    nc.compile()
    return nc


def _split16(a):
    hi = a.astype(np.float16)
    lo = (a - hi.astype(np.float32)).astype(np.float16)
    return hi, lo


def _pack_w(a16):
    # [768, N] fp16 -> [128, 6*N]: out[p, k*N+c] = a16[k*128+p, c]
    N = a16.shape[1]
    return np.ascontiguousarray(
        a16.reshape(ND, 128, N).transpose(1, 0, 2).reshape(128, ND * N))


def _prep_weights(Wq, bq, Wk, bk, Wv, bv, Wo, bo, has_bias):
    f32 = np.float32
    wq = np.asarray(Wq, f32) * f32(0.125)
    wk = np.asarray(Wk, f32)
    wq_hi, wq_lo = _split16(wq)
    wk_hi, wk_lo = _split16(wk)
    w = {
        "wq_hi": _pack_w(wq_hi), "wq_lo": _pack_w(wq_lo),
        "wk_hi": _pack_w(wk_hi), "wk_lo": _pack_w(wk_lo),
        "wv": _pack_w(np.asarray(Wv, f32).astype(np.float16)),
        "wo": _pack_w(np.asarray(Wo, f32).astype(np.float16)),
    }
    if has_bias:
        w["bq"] = (np.asarray(bq, f32) * f32(0.125)).astype(
            np.float16).reshape(1, D)
        w["bk"] = np.asarray(bk, f32).astype(np.float16).reshape(1, D)
        w["bv"] = np.asarray(bv, f32).astype(np.float16).reshape(1, D)
        w["bo"] = np.asarray(bo, f32).astype(np.float16).reshape(1, D)
    return w


def _pack_x(xc16):
    # [R, 768] fp16 -> [128, 6*RPAD]: out[p, k*RPAD+j] = x16[j, k*128+p]
    xt = np.zeros((D, RPAD), np.float16)
    xt[:, :R] = xc16.T
    return np.ascontiguousarray(
        xt.reshape(ND, 128, RPAD).transpose(1, 0, 2).reshape(128, ND * RPAD))


def _make_in_maps(x, w):
    x = np.asarray(x, np.float32)
    in_maps = []
    for c in range(NCORES):
        m = dict(w)
        xc = x[c * BL:(c + 1) * BL].reshape(R, D)
        x16 = xc.astype(np.float16)
        m["x16p"] = _pack_x(x16)
        if N_TERMS == 3:
            m["xlop"] = _pack_x(
                (xc - x16.astype(np.float32)).astype(np.float16))
        in_maps.append(m)
    return in_maps


def kernel(x, Wq, bq, Wk, bk, Wv, bv, Wo, bo):
    from concourse import bass_utils

    has_bias = any(float(np.abs(np.asarray(v)).max()) != 0.0
                   for v in (bq, bk, bv, bo))
    key = ("nc", has_bias, N_TERMS)
    if key not in _CACHE:
        _CACHE[key] = _build(has_bias, N_TERMS)
    nc = _CACHE[key]

    w = _prep_weights(Wq, bq, Wk, bk, Wv, bv, Wo, bo, has_bias)
    in_maps = _make_in_maps(x, w)

    res = bass_utils.run_bass_kernel_spmd(nc, in_maps, list(range(NCORES)))
    out = np.concatenate(
        [res.results[c]["out"].reshape(BL, T, D) for c in range(NCORES)],
        axis=0)
    return out.astype(np.float32)


# revision 9
# speedup vs baseline: 1.6535x; 1.0270x over previous
"""Multi-head attention forward on 8 TRN2 NeuronCores (data-parallel over batch).

Reference computation (B=64, T=197, D=768, H=12, DK=64, fp32):
    q = split_heads(x @ Wq + bq); k = ...; v = ...
    scores = floor((q @ k^T) / 8); attn = softmax(scores); out = attn @ v
    return merge_heads(out) @ Wo + bo

Numerics: floor() before softmax makes the Q/K path sensitive.  q/k
projections run as 2-term fp16 matmuls: W is split hi+lo (22-bit
mantissa), x is truncated to fp16 (the x_lo term is dropped):
q = x16 @ W_hi + x16 @ W_lo, exact fp32 PSUM accumulation.  Measured
rel err vs the fp32 reference: 1.5e-2 (budget 2e-2); the 3-term
variant (adds x_lo @ W_hi, rel err 1.2e-3) is kept behind N_TERMS=3.
The scores matmul is native fp32, two heads row-packed via
tile_position (packed pairs execute concurrently on the PE).  The V
path (v proj, attn@v, out proj) runs in plain fp16.

Layout: x is transposed + fp16-cast + partition-packed on the HOST
(x16p[p, k*1584+j] = x16[j, k*128+p]) and weights partition-packed
(w[p, k*768+c] = W[k*128+p, c]) so every DMA descriptor is a 9-19KB
contiguous run — the on-chip transpose phase and its 256B-descriptor
DMA-transpose storm are gone entirely (input DMA ~7us, was ~50us).

Schedule: one software-pipelined instruction stream so the PE never
idles (idle >3.4us re-throttles the PE clock to 1.2GHz via HAM).
Projections run in 4 column chunks of 394 rows = one batch pair each.
Attention for pair p (scores -> floor (DVE magic-number round) -> Exp
(ScalarE, -MAGIC folded into the bias) -> attn@v -> normalize -> out
proj) is woven into projection chunk p+1's matmul stream; v-proj units
(dependency-free) pad the latency-sensitive spots (reciprocal ->
broadcast).  Pair 3's scores are pulled into iteration 3 so the tail
is short.  attn@v right-appends a ones column per head (v stride 65)
so the softmax denominator falls out of the same matmul; denominators
are gathered, one batched reciprocal, PE-broadcast, DVE multiply.  PE
warmup matmuls + Exp-table preload run during the initial DMA wait.

All PSUM tiles come from one shared-tag pool (8 banks round-robin).
"""

import numpy as np

B, T, D, H, DK = 64, 197, 768, 12, 64
NCORES = 8
BL = B // NCORES          # 8 batch elements per core
R = BL * T                # 1576 rows per core
RPAD = 1584               # row count padded (keeps host packing regular)
ND = D // 128             # 6 chunks of 128 along D
NC4 = 4                   # proj col chunks (one batch pair each)
CW = R // NC4             # 394 = 2*T
HV = DK + 1               # 65: per-head v stride (ones column at 64)
KEYCHUNKS = [(0, 128), (128, 69)]
MAGIC = float(3 * 2 ** 22)  # 1.5*2^23: x-0.5+MAGIC stays in [2^23,2^24), ulp=1
N_TERMS = 2               # 2: q/k = x16@W_hi + x16@W_lo; 3: + xlo@W_hi

_CACHE = {}


def _build(has_bias, n_terms):
    import concourse.bacc as bacc
    import concourse.mybir as mybir
    import concourse.tile as tile

    f32 = mybir.dt.float32
    f16 = mybir.dt.float16
    AF = mybir.ActivationFunctionType
    OP = mybir.AluOpType

    nc = bacc.Bacc("TRN2", target_bir_lowering=False, debug=False,
                   num_devices=NCORES)

    x16_d = nc.dram_tensor("x16p", [128, ND * RPAD], f16,
                           kind="ExternalInput").ap()
    if n_terms == 3:
        xlo_d = nc.dram_tensor("xlop", [128, ND * RPAD], f16,
                               kind="ExternalInput").ap()
    w_d = {}
    for nm in ("wq_hi", "wq_lo", "wk_hi", "wk_lo", "wv", "wo"):
        w_d[nm] = nc.dram_tensor(nm, [128, ND * D], f16,
                                 kind="ExternalInput").ap()
    if has_bias:
        b_d = {nm: nc.dram_tensor(nm, [1, D], f16, kind="ExternalInput").ap()
               for nm in ("bq", "bk", "bv", "bo")}
    out_d = nc.dram_tensor("out", [R, D], f32, kind="ExternalOutput").ap()

    with tile.TileContext(nc) as tc:
        with tc.tile_pool(name="static", bufs=1) as Ps, \
             tc.tile_pool(name="work", bufs=1) as Pw, \
             tc.tile_pool(name="psum", bufs=8, space="PSUM") as Pp:

            def ptile(nm):
                return Pp.tile([128, CW], f32, name=nm, tag="ps", bufs=8,
                               uniquify=True)

            xall = Ps.tile([128, ND * RPAD], f16, name="xall")
            if n_terms == 3:
                xloall = Ps.tile([128, ND * RPAD], f16, name="xloall")
            wsb = {nm: Ps.tile([128, ND * D], f16, name=nm)
                   for nm in ("wq_hi", "wq_lo", "wk_hi", "wk_lo", "wv", "wo")}
            # v16e[2b+kc][keys<=128, 12*65]; col h*65+64 holds ones
            v16e = [Ps.tile([128, H * HV], f16, name=f"v16e_{i}")
                    for i in range(2 * BL)]
            ones_row = Ps.tile([128, CW], f16, name="ones_row")
            negmagic = Ps.tile([128, 1], f32, name="negmagic")
            prime = Ps.tile([1, 1], f16, name="prime")
            if has_bias:
                bsb = {nm: Ps.tile([1, D], f16, name=f"{nm}_sb")
                       for nm in ("bq", "bk", "bv", "bo")}

            def xs(k, c0, ln):
                return xall[:, k * RPAD + c0:k * RPAD + c0 + ln]

            def xls(k, c0, ln):
                return xloall[:, k * RPAD + c0:k * RPAD + c0 + ln]

            def ws(nm, k, c0, ln):
                return wsb[nm][:, k * D + c0:k * D + c0 + ln]

            # ---- no-DMA-dependency setup: memsets, engine warmups ----
            nc.vector.memset(ones_row, 1.0)
            nc.vector.memset(negmagic, -MAGIC)
            for i in range(2 * BL):
                onescol = v16e[i].rearrange("p (h c) -> p h c",
                                            c=HV)[:, :, DK:DK + 1]
                nc.gpsimd.memset(onescol, 1.0)
            # Exp table preload on ScalarE (one-time 1.3us table load)
            nc.scalar.activation(prime, ones_row[:1, :1], AF.Exp,
                                 bias=negmagic[:1, :1])
            # PE warmup: keep HAM at full clock until real work arrives
            for i in range(34):
                wu = ptile("wu")
                nc.tensor.matmul(wu, ones_row[:, :128], ones_row,
                                 start=True, stop=True)

            # ---- DMAs (all large contiguous descriptors) ----
            # x split by column range so chunks 0-1 (cols < 800) gate the
            # first projections on ~1.2MB instead of the full 2.4MB
            XSP = 800
            x3 = xall.rearrange("p (k j) -> p k j", k=ND)
            xd3 = x16_d.rearrange("p (k j) -> p k j", k=ND)
            nc.sync.dma_start(x3[:, :, :XSP], xd3[:, :, :XSP])
            nc.sync.dma_start(wsb["wq_hi"], w_d["wq_hi"])
            nc.sync.dma_start(wsb["wq_lo"], w_d["wq_lo"])
            nc.sync.dma_start(x3[:, :, XSP:], xd3[:, :, XSP:])
            if n_terms == 3:
                nc.sync.dma_start(xloall, xlo_d)
            for nm in ("wk_hi", "wk_lo", "wv", "wo"):
                nc.sync.dma_start(wsb[nm], w_d[nm])
            if has_bias:
                for nm in ("bq", "bk", "bv", "bo"):
                    nc.sync.dma_start(bsb[nm], b_d[nm])

            # ---- stage helpers (each call ISSUES instructions) ----
            qT = {}   # (proj, c, n) -> sbuf tile [128, CW] f32
            eTs = {}  # (b, hp) -> [e_t hl0, e_t hl1]
            otfs = {}  # (b, hp) -> otf tile
            oT16s = {}  # (b, hp) -> oT16 tile
            dns = {}
            rd16s = {}

            def qk_tile(c, proj, n):
                whi, wlo, b_nm = (("wq_hi", "wq_lo", "bq") if proj == "q"
                                  else ("wk_hi", "wk_lo", "bk"))
                c0 = c * CW
                pp = ptile("pp")
                for k in range(ND):
                    nc.tensor.matmul(pp, ws(whi, k, n * 128, 128),
                                     xs(k, c0, CW), start=(k == 0),
                                     stop=False)
                for k in range(ND):
                    last = (k == ND - 1 and n_terms == 2 and not has_bias)
                    nc.tensor.matmul(pp, ws(wlo, k, n * 128, 128),
                                     xs(k, c0, CW), start=False, stop=last)
                if n_terms == 3:
                    for k in range(ND):
                        last = (k == ND - 1 and not has_bias)
                        nc.tensor.matmul(pp, ws(whi, k, n * 128, 128),
                                         xls(k, c0, CW),
                                         start=False, stop=last)
                if has_bias:
                    nc.tensor.matmul(pp, bsb[b_nm][:1, n * 128:n * 128 + 128],
                                     ones_row[:1, :CW],
                                     start=False, stop=True)
                dst = Pw.tile([128, CW], f32, name=f"{proj}T", tag=f"{proj}T",
                              bufs=12, uniquify=True)
                nc.scalar.activation(dst, pp, AF.Copy)
                qT[(proj, c, n)] = dst

            def vp_unit(b, j):
                kc, half = j // 2, j % 2
                koff, klen = KEYCHUNKS[kc]
                base = b * T
                c0 = half * 384
                vp = ptile("vp")
                vps = vp[:klen, :384]
                for d in range(ND):
                    nc.tensor.matmul(
                        vps, xs(d, base + koff, klen), ws("wv", d, c0, 384),
                        start=(d == 0),
                        stop=(d == ND - 1 and not has_bias))
                if has_bias:
                    nc.tensor.matmul(vps, ones_row[:1, :klen],
                                     bsb["bv"][:1, c0:c0 + 384],
                                     start=False, stop=True)
                dst = v16e[2 * b + kc]
                dst3 = dst[:klen, :].rearrange("p (h c) -> p h c",
                                               c=HV)[:, :, 0:DK]
                nc.scalar.activation(
                    dst3[:, half * 6:(half + 1) * 6, :],
                    vps.rearrange("p (h c) -> p h c", c=DK), AF.Copy)

            def sc_unit(b, hp):
                c = b // 2
                qoff = (b % 2) * T
                eT = []
                for hl in range(2):
                    pb = 64 * hl
                    sc = ptile("sc")
                    for kc, (koff, klen) in enumerate(KEYCHUNKS):
                        nc.tensor.matmul(
                            sc[:klen, kc * T:(kc + 1) * T],
                            qT[("k", c, hp)][pb:pb + 64,
                                             qoff + koff:qoff + koff + klen],
                            qT[("q", c, hp)][pb:pb + 64, qoff:qoff + T],
                            start=True, stop=True, tile_position=(pb, 0))
                    fl = Pw.tile([128, 2 * T], f32, name="fl", tag="fl",
                                 bufs=6, uniquify=True)
                    nc.vector.tensor_scalar(fl, sc, -0.5, MAGIC,
                                            OP.add, OP.add)
                    e_t = Pw.tile([128, 2 * T], f16, name="e_t", tag="eT",
                                  bufs=16, uniquify=True)
                    nc.scalar.activation(e_t, fl, AF.Exp,
                                         bias=negmagic[:, :1])
                    eT.append(e_t)
                eTs[(b, hp)] = eT

            def av_unit(b, hp):
                eT = eTs.pop((b, hp))
                if hp == 0:
                    # denominator gather split in two so the reciprocal can
                    # start after hp 0-3 (heads 0-7) instead of after all 12
                    dns[b] = (
                        Pw.tile([128, 2 * T], f32, name="dnA", tag="dnA",
                                bufs=3, uniquify=True),
                        Pw.tile([128, T], f32, name="dnB", tag="dnB",
                                bufs=3, uniquify=True))
                dnA, dnB = dns[b]
                otf = Pw.tile([128, T], f32, name="otf", tag="otf",
                              bufs=14, uniquify=True)
                op_ = ptile("oT")
                for hl in range(2):
                    h = 2 * hp + hl
                    for kc, (koff, klen) in enumerate(KEYCHUNKS):
                        nc.tensor.matmul(
                            op_[0:HV, hl * T:(hl + 1) * T],
                            v16e[2 * b + kc][:klen, h * HV:(h + 1) * HV],
                            eT[hl][:klen, kc * T:(kc + 1) * T],
                            start=(kc == 0), stop=(kc == len(KEYCHUNKS) - 1))
                    pbase = 32 * (h % 4)
                    dn, cb = (dnA, (h // 4) * T) if h < 8 else (dnB, 0)
                    nc.vector.tensor_copy(dn[pbase:pbase + 1, cb:cb + T],
                                          op_[64:65, hl * T:(hl + 1) * T])
                    if hl == 0:
                        nc.scalar.activation(otf[0:64, :], op_[0:64, :T],
                                             AF.Copy)
                    else:
                        nc.vector.tensor_copy(otf[64:128, :],
                                              op_[0:64, T:2 * T])
                otfs[(b, hp)] = otf

            def recip_unit(b, part):
                dnA, dnB = dns[b]
                # the fp16 cast runs on idle GpSimd in steady state, but on
                # DVE for the tail pair where the rd16->bcast latency matters
                cp = nc.vector.tensor_copy if b >= 6 else nc.gpsimd.tensor_copy
                if part == 0:
                    rdf = Pw.tile([128, 2 * T], f32, name="rdfA", tag="rdfA",
                                  bufs=2, uniquify=True)
                    rd16 = Pw.tile([128, 2 * T], f16, name="rdA", tag="rdA",
                                   bufs=3, uniquify=True)
                    nc.vector.reciprocal_approx_fast(rdf, dnA)
                    cp(rd16, rdf)
                    rd16s[b] = [rd16, None]
                else:
                    rdf = Pw.tile([128, T], f32, name="rdfB", tag="rdfB",
                                  bufs=2, uniquify=True)
                    rd16 = Pw.tile([128, T], f16, name="rdB", tag="rdB",
                                   bufs=3, uniquify=True)
                    nc.vector.reciprocal_approx_fast(rdf, dnB)
                    cp(rd16, rdf)
                    rd16s[b][1] = rd16
                    del dns[b]

            def bc_unit(b, hp):
                bc = ptile("bc")
                for hl in range(2):
                    h = 2 * hp + hl
                    pbase = 32 * (h % 4)
                    rd16, cb = ((rd16s[b][0], (h // 4) * T) if h < 8
                                else (rd16s[b][1], 0))
                    nc.tensor.matmul(
                        bc[64 * hl:64 * hl + 64, :T],
                        ones_row[pbase:pbase + 1, :64],
                        rd16[pbase:pbase + 1, cb:cb + T],
                        start=True, stop=True,
                        tile_position=(pbase, 64 * hl))
                oT16 = Pw.tile([128, T], f16, name="oT16", tag="oT16",
                               bufs=14, uniquify=True)
                nc.vector.tensor_tensor(oT16, otfs.pop((b, hp)), bc[:, :T],
                                        OP.mult)
                oT16s[(b, hp)] = oT16

            def op_unit(b, rc):
                base = b * T
                roff, rn = (0, 128) if rc == 0 else (128, T - 128)
                fs = Pw.tile([128, D], f32, name="fs", tag="fs",
                             bufs=4, uniquify=True)
                for half in range(2):
                    c0 = half * 384
                    fp_ = ptile("fp")
                    for d in range(ND):
                        nc.tensor.matmul(
                            fp_[:rn, :384],
                            oT16s[(b, d)][:, roff:roff + rn],
                            ws("wo", d, c0, 384),
                            start=(d == 0),
                            stop=(d == ND - 1 and not has_bias))
                    if has_bias:
                        nc.tensor.matmul(fp_[:rn, :384], ones_row[:1, :rn],
                                         bsb["bo"][:1, c0:c0 + 384],
                                         start=False, stop=True)
                    nc.scalar.activation(fs[:rn, c0:c0 + 384],
                                         fp_[:rn, :384], AF.Copy)
                    nc.sync.dma_start(
                        out_d[base + roff:base + roff + rn, c0:c0 + 384],
                        fs[:rn, c0:c0 + 384])
                if rc == 1:
                    for d in range(ND):
                        del oT16s[(b, d)]

            # ---- pipelined program ----
            # c0 prologue: projections chunk 0 + v for pair 0
            for n in range(ND):
                qk_tile(0, "q", n)
            for n in range(ND):
                qk_tile(0, "k", n)
            for j in range(4):
                vp_unit(0, j)
            for j in range(4):
                vp_unit(1, j)

            # iterations c=1..3: attention(pair c-1) woven into chunk c
            for c in range(1, NC4):
                B0, B1 = 2 * (c - 1), 2 * (c - 1) + 1
                C0, C1 = 2 * c, 2 * c + 1
                qkt = [(p, n) for p in ("q", "k") for n in range(ND)]

                sc_unit(B0, 0); sc_unit(B0, 1)
                qk_tile(c, *qkt[0])
                sc_unit(B0, 2); av_unit(B0, 0)
                qk_tile(c, *qkt[1])
                sc_unit(B0, 3); av_unit(B0, 1)
                qk_tile(c, *qkt[2])
                sc_unit(B0, 4); av_unit(B0, 2)
                qk_tile(c, *qkt[3])
                sc_unit(B0, 5); av_unit(B0, 3)
                recip_unit(B0, 0)
                qk_tile(c, *qkt[4])
                av_unit(B0, 4); av_unit(B0, 5)
                recip_unit(B0, 1)
                qk_tile(c, *qkt[5])
                sc_unit(B1, 0); sc_unit(B1, 1)
                qk_tile(c, *qkt[6])
                sc_unit(B1, 2); av_unit(B1, 0)
                bc_unit(B0, 0); bc_unit(B0, 1)
                qk_tile(c, *qkt[7])
                sc_unit(B1, 3); av_unit(B1, 1)
                bc_unit(B0, 2); bc_unit(B0, 3)
                qk_tile(c, *qkt[8])
                sc_unit(B1, 4); av_unit(B1, 2)
                bc_unit(B0, 4); bc_unit(B0, 5)
                qk_tile(c, *qkt[9])
                sc_unit(B1, 5); av_unit(B1, 3)
                recip_unit(B1, 0)
                op_unit(B0, 0)
                qk_tile(c, *qkt[10])
                av_unit(B1, 4)
                op_unit(B0, 1)
                qk_tile(c, *qkt[11])
                av_unit(B1, 5)
                recip_unit(B1, 1)
                if c < 3:
                    vp_unit(C0, 0); vp_unit(C0, 1)
                    bc_unit(B1, 0); bc_unit(B1, 1); bc_unit(B1, 2)
                    vp_unit(C0, 2); vp_unit(C0, 3)
                    bc_unit(B1, 3); bc_unit(B1, 4); bc_unit(B1, 5)
                    vp_unit(C1, 0); vp_unit(C1, 1)
                    op_unit(B1, 0)
                    vp_unit(C1, 2); vp_unit(C1, 3)
                    op_unit(B1, 1)
                else:
                    # extended weave: pair-3 scores start here (chunk 3 done)
                    sc_unit(6, 0); sc_unit(6, 1)
                    vp_unit(6, 0); vp_unit(6, 1)
                    bc_unit(5, 0); bc_unit(5, 1); bc_unit(5, 2)
                    sc_unit(6, 2); sc_unit(6, 3)
                    vp_unit(6, 2); vp_unit(6, 3)
                    bc_unit(5, 3); bc_unit(5, 4); bc_unit(5, 5)
                    sc_unit(6, 4); sc_unit(6, 5)
                    vp_unit(7, 0); vp_unit(7, 1)
                    op_unit(5, 0)
                    sc_unit(7, 0); av_unit(6, 0)
                    vp_unit(7, 2); vp_unit(7, 3)
                    op_unit(5, 1)
                    sc_unit(7, 1); av_unit(6, 1)
                    sc_unit(7, 2); av_unit(6, 2)
                    sc_unit(7, 3); av_unit(6, 3)
                    recip_unit(6, 0)
                    sc_unit(7, 4); av_unit(6, 4)
                    sc_unit(7, 5); av_unit(6, 5)
                    recip_unit(6, 1)
                    av_unit(7, 0); av_unit(7, 1)
                    bc_unit(6, 0); bc_unit(6, 1); bc_unit(6, 2)
                    av_unit(7, 2); av_unit(7, 3)
                    recip_unit(7, 0)
                    bc_unit(6, 3); bc_unit(6, 4); bc_unit(6, 5)
                    av_unit(7, 4); av_unit(7, 5)
                    recip_unit(7, 1)
                    op_unit(6, 0)
                    bc_unit(7, 0); bc_unit(7, 1); bc_unit(7, 2)
                    bc_unit(7, 3)
                    op_unit(6, 1)
                    bc_unit(7, 4); bc_unit(7, 5)
                    op_unit(7, 0)
                    op_unit(7, 1)

    nc.compile()
    return nc


def _split16(a):
    hi = a.astype(np.float16)
    lo = (a - hi.astype(np.float32)).astype(np.float16)
    return hi, lo


def _pack_w(a16):
    # [768, N] fp16 -> [128, 6*N]: out[p, k*N+c] = a16[k*128+p, c]
    N = a16.shape[1]
    return np.ascontiguousarray(
        a16.reshape(ND, 128, N).transpose(1, 0, 2).reshape(128, ND * N))


def _prep_weights(Wq, bq, Wk, bk, Wv, bv, Wo, bo, has_bias):
    f32 = np.float32
    wq = np.asarray(Wq, f32) * f32(0.125)
    wk = np.asarray(Wk, f32)
    wq_hi, wq_lo = _split16(wq)
    wk_hi, wk_lo = _split16(wk)
    w = {
        "wq_hi": _pack_w(wq_hi), "wq_lo": _pack_w(wq_lo),
        "wk_hi": _pack_w(wk_hi), "wk_lo": _pack_w(wk_lo),
        "wv": _pack_w(np.asarray(Wv, f32).astype(np.float16)),
        "wo": _pack_w(np.asarray(Wo, f32).astype(np.float16)),
    }
    if has_bias:
        w["bq"] = (np.asarray(bq, f32) * f32(0.125)).astype(
            np.float16).reshape(1, D)
        w["bk"] = np.asarray(bk, f32).astype(np.float16).reshape(1, D)
        w["bv"] = np.asarray(bv, f32).astype(np.float16).reshape(1, D)
        w["bo"] = np.asarray(bo, f32).astype(np.float16).reshape(1, D)
    return w


def _pack_x(xc16):
    # [R, 768] fp16 -> [128, 6*RPAD]: out[p, k*RPAD+j] = x16[j, k*128+p]
    xt = np.zeros((D, RPAD), np.float16)
    xt[:, :R] = xc16.T
    return np.ascontiguousarray(
        xt.reshape(ND, 128, RPAD).transpose(1, 0, 2).reshape(128, ND * RPAD))


def _make_in_maps(x, w):
    x = np.asarray(x, np.float32)
    in_maps = []
    for c in range(NCORES):
        m = dict(w)
        xc = x[c * BL:(c + 1) * BL].reshape(R, D)
        x16 = xc.astype(np.float16)
        m["x16p"] = _pack_x(x16)
        if N_TERMS == 3:
            m["xlop"] = _pack_x(
                (xc - x16.astype(np.float32)).astype(np.float16))
        in_maps.append(m)
    return in_maps


def kernel(x, Wq, bq, Wk, bk, Wv, bv, Wo, bo):
    from concourse import bass_utils

    has_bias = any(float(np.abs(np.asarray(v)).max()) != 0.0
                   for v in (bq, bk, bv, bo))
    key = ("nc", has_bias, N_TERMS)
    if key not in _CACHE:
        _CACHE[key] = _build(has_bias, N_TERMS)
    nc = _CACHE[key]

    w = _prep_weights(Wq, bq, Wk, bk, Wv, bv, Wo, bo, has_bias)
    in_maps = _make_in_maps(x, w)

    res = bass_utils.run_bass_kernel_spmd(nc, in_maps, list(range(NCORES)))
    out = np.concatenate(
        [res.results[c]["out"].reshape(BL, T, D) for c in range(NCORES)],
        axis=0)
    return out.astype(np.float32)
